# revision 1
# baseline (speedup 1.0000x reference)
"""Bass/Trainium2 kernel for nn_Net_40063454937541 (CurvGN 2-layer GNN).

Strategy (8 NeuronCores, SPMD single program):
  - Node space [100000] split into 8 contiguous ranges of 12500.
  - Exploits w_mul >= 0: leaky_relu(w_mul @ ma) is exactly linear in w_mul, so
    pre-softmax edge logits are affine in w_mul and the bias cancels in the
    segment softmax: softmax weights = exp(w_e * u[c]) / sum_src exp(w * u[c]).
  - Phase A (edges sharded by src range): per-source denominators D1[104] via
    one-hot matmul segment sums, scatter-added into node-sharded D table.
  - h1 phase: h1 = x @ w1 + b1 (host-pretransposed x), P1 = h1/(D1+eps),
    AllGather -> replicated P1 table [100000, 64].
  - Phase B1 (edges sharded by dst range): dma_gather P1[src] (mod-4 stride
    trick for int16 index range), msg = exp(w*u1)*P1[src], one-hot matmul
    segment sum by dst -> out1 slab; fused epilogue ELU -> @w2 -> /D2 -> P2;
    AllGather P2.
  - Phase B2: same machinery with u2 on P2 table -> log_softmax -> output.
All floating point math runs on device; host only shards/sorts/packs indices
and folds the tiny (64-wide) weight MLPs.
"""

import numpy as np

N_NODES = 100000
N_EDGES = 1600000
N_FEAT = 500
HID = 64
N_CLS = 40

NCORES = 8
NLOC = N_NODES // NCORES          # 12500
P = 128
T_B = 16                          # edge tiles per B supertile (4 classes x 4)
CLS_TILES = 4                     # tiles per mod-4 class region
CAP_CLASS = CLS_TILES * P         # 512 edge slots per class region
T_A = 16                          # edge tiles per A supertile
CAP_A = T_A * P                   # 2048
G8 = 8                            # supertiles per scatter/epilogue batch
SEG_PAD = 200.0                   # one-hot never matches
TRASH = NLOC                      # trash row index (12500)
NROWS = NLOC + 44                 # 12544 rows (98*128) for node tables
NT_H1 = 98                        # node tiles for h1 phase (98*128 = 12544)
EPS = 1e-16


def _wrap16(vals, dtype=np.int16):
    """Layout an index vector [n] (n % 16 == 0) into the q7 wrapped form
    [128, n//16]: position i lives at [i % 16, i // 16], replicated in all
    8 groups of 16 partitions."""
    v = np.asarray(vals)
    n = v.shape[0]
    assert n % 16 == 0
    w = v.reshape(n // 16, 16).T.astype(dtype)     # [16, n//16]
    return np.tile(w, (8, 1))                      # [128, n//16]


def _pack_nodes(counts_list, max_nodes, caps):
    """Greedy pack nodes (in order) into supertiles.
    counts_list: [n_classes][n_nodes] per-class edge counts.
    caps: per-class slot capacity. Returns list of (start, n_nodes)."""
    n_nodes = len(counts_list[0])
    n_cls = len(counts_list)
    out = []
    start = 0
    used = [0] * n_cls
    nn = 0
    for i in range(n_nodes):
        c = [counts_list[r][i] for r in range(n_cls)]
        fits = nn < max_nodes and all(used[r] + c[r] <= caps[r] for r in range(n_cls))
        if not fits:
            out.append((start, nn))
            start = i
            used = [0] * n_cls
            nn = 0
        for r in range(n_cls):
            used[r] += c[r]
            assert used[r] <= caps[r], "single node exceeds class capacity"
        nn += 1
    out.append((start, nn))
    return out


def _fold_weights(m1a, m1b_w, m1b_b, m2a, m2b_w, m2b_b, w2, b2):
    """Fold the tiny weight-MLPs using w_mul >= 0 (leaky_relu linear in w)."""
    s1 = np.where(m1a[0] >= 0, m1a[0], 0.2 * m1a[0])   # [64]
    u1 = (s1 @ m1b_w).astype(np.float32)               # [64]
    s2 = np.where(m2a[0] >= 0, m2a[0], 0.2 * m2a[0])   # [40]
    u2 = (s2 @ m2b_w).astype(np.float32)               # [40]
    # ELU fold: elu(x) = relu(x) + exp(min(x,0)) - 1; (q-1)@w2+b2 = q@w2+b2p
    b2p = (b2 - w2.sum(axis=0)).astype(np.float32)     # [40]
    return u1, u2, b2p


def _preprocess(inputs):
    """Build all per-core host arrays. Integer/layout work only (plus the
    tiny 64-wide weight folds)."""
    src = np.asarray(inputs["edge_index"][0], dtype=np.int64)
    dst = np.asarray(inputs["edge_index"][1], dtype=np.int64)
    w = np.asarray(inputs["w_mul"], dtype=np.float32).reshape(-1)
    x = np.asarray(inputs["x"], dtype=np.float32)

    u1, u2, b2p = _fold_weights(
        np.asarray(inputs["m1a"], np.float32), np.asarray(inputs["m1b_w"], np.float32),
        np.asarray(inputs["m1b_b"], np.float32), np.asarray(inputs["m2a"], np.float32),
        np.asarray(inputs["m2b_w"], np.float32), np.asarray(inputs["m2b_b"], np.float32),
        np.asarray(inputs["w2"], np.float32), np.asarray(inputs["b2"], np.float32))
    assert np.abs(u1).max() < 8 and np.abs(u2).max() < 8

    ucat = np.concatenate([u1, u2])                    # [104]
    u2pad = np.zeros(64, np.float32)
    u2pad[:N_CLS] = u2
    w2pad = np.zeros((64, 64), np.float32)
    w2pad[:, :N_CLS] = np.asarray(inputs["w2"], np.float32)
    b2ppad = np.zeros(64, np.float32)
    b2ppad[:N_CLS] = b2p

    cores = []
    for i in range(NCORES):
        lo, hi = i * NLOC, (i + 1) * NLOC
        core = {}

        # ---------- phase A (by src) ----------
        m = (src >= lo) & (src < hi)
        es, ew = src[m] - lo, w[m]
        order = np.argsort(es, kind="stable")
        es, ew = es[order], ew[order]
        cnt = np.bincount(es, minlength=NLOC)
        sts = _pack_nodes([cnt], P, [CAP_A])
        SA = len(sts)
        edge_starts = np.concatenate([[0], np.cumsum(cnt)])
        wA = np.zeros((SA, P, T_A), np.float32)
        segA = np.full((SA, P, T_A), SEG_PAD, np.float32)
        rowsA = np.full((SA, P), TRASH, np.int64)
        for s, (n0, k) in enumerate(sts):
            e0, e1 = edge_starts[n0], edge_starts[n0 + k]
            ne = e1 - e0
            q = np.arange(ne)
            kk, pp = q // P, q % P
            wA[s, pp, kk] = ew[e0:e1]
            segA[s, pp, kk] = (es[e0:e1] - n0).astype(np.float32)
            rowsA[s, :k] = n0 + np.arange(k)
        core["SA"] = SA
        core["wsegA"] = np.concatenate([wA, segA], axis=2)  # [SA,128,32]
        core["rowsA"] = rowsA

        # ---------- phase B (by dst) ----------
        m = (dst >= lo) & (dst < hi)
        ed, eg, ew = dst[m] - lo, src[m], w[m]
        cls = (eg % 4).astype(np.int64)
        order = np.lexsort((cls, ed))        # sort by dst, then class
        ed, eg, ew, cls = ed[order], eg[order], ew[order], cls[order]
        cntr = [np.bincount(ed[cls == r], minlength=NLOC) for r in range(4)]
        sts = _pack_nodes(cntr, P, [CAP_CLASS] * 4)
        SB = len(sts)
        # per-class views, each sorted by dst
        per_r = [(ed[cls == r], eg[cls == r], ew[cls == r]) for r in range(4)]
        startr = [np.concatenate([[0], np.cumsum(cntr[r])]) for r in range(4)]
        wB = np.zeros((SB, P, T_B), np.float32)
        segB = np.full((SB, P, T_B), SEG_PAD, np.float32)
        rowsB = np.full((SB, P), TRASH, np.int64)
        gidx = np.zeros((SB, 4, CAP_CLASS), np.int64)   # per supertile+class
        for s, (n0, k) in enumerate(sts):
            rowsB[s, :k] = n0 + np.arange(k)
            for r in range(4):
                a, b = startr[r][n0], startr[r][n0 + k]
                ne = b - a
                assert ne <= CAP_CLASS
                edr, egr, ewr = per_r[r]
                q = np.arange(ne)
                kk, pp = q // P, q % P
                wB[s, pp, 4 * r + kk] = ewr[a:b]
                segB[s, pp, 4 * r + kk] = (edr[a:b] - n0).astype(np.float32)
                gidx[s, r, :ne] = egr[a:b] // 4
        core["SB"] = SB
        core["wsegB"] = np.concatenate([wB, segB], axis=2)  # [SB,128,32]
        core["rowsB"] = rowsB
        core["gidxB"] = gidx

        # ---------- h1 phase: x pretransposed ----------
        xp = np.zeros((NT_H1, 4, 125, P), np.float32)
        xl = x[lo:hi]                                      # [12500, 500]
        xpad = np.zeros((NROWS, N_FEAT), np.float32)
        xpad[:NLOC] = xl
        for j in range(NT_H1):
            blk = xpad[j * P:(j + 1) * P]                  # [128, 500]
            xp[j] = blk.T.reshape(4, 125, P)
        core["xpre"] = xp
        cores.append(core)

    # pad supertile counts to the max (+ multiple of G8) across cores
    SA_max = -(-max(c["SA"] for c in cores) // G8) * G8
    SB_max = -(-max(c["SB"] for c in cores) // G8) * G8
    for c in cores:
        c["wsegA"] = _pad_st(c["wsegA"], SA_max)
        c["rowsA"] = _pad_rows(c["rowsA"], SA_max)
        c["wsegB"] = _pad_st(c["wsegB"], SB_max)
        c["rowsB"] = _pad_rows(c["rowsB"], SB_max)
        g = np.zeros((SB_max, 4, CAP_CLASS), np.int64)
        g[:c["SB"]] = c["gidxB"]
        c["gidxB"] = g

    # wrapped int16 arrays
    for c in cores:
        # gather idx per pair g, class r: [SB/2? -> G2 groups][4][1024]
        gi = c["gidxB"]                                    # [SB,4,512]
        pairs = gi.reshape(SB_max // 2, 2, 4, CAP_CLASS).transpose(0, 2, 1, 3)
        pairs = pairs.reshape(SB_max // 2, 4, 2 * CAP_CLASS)   # [G2,4,1024]
        c["gidx16"] = np.stack([
            np.concatenate([_wrap16(pairs[g, r]) for r in range(4)], axis=1)
            for g in range(SB_max // 2)])                  # [G2,128,256] i16
        c["rowsA16"] = np.stack([
            _wrap16(c["rowsA"][h * G8:(h + 1) * G8].reshape(-1))
            for h in range(SA_max // G8)])                 # [GA8,128,64] i16
        c["rowsB16"] = np.stack([
            _wrap16(c["rowsB"][h * G8:(h + 1) * G8].reshape(-1))
            for h in range(SB_max // G8)])                 # [GB8,128,64] i16

    consts = {
        "u1": u1, "u2pad": u2pad, "ucat": ucat, "w2pad": w2pad,
        "b2ppad": b2ppad,
        "b1": np.asarray(inputs["b1"], np.float32),
        "w1": np.asarray(inputs["w1"], np.float32),
    }
    return {"cores": cores, "SA": SA_max, "SB": SB_max, "consts": consts}


def _pad_st(a, S):
    out = np.zeros((S,) + a.shape[1:], a.dtype)
    if a.dtype == np.float32:
        out[:, :, a.shape[2] // 2:] = SEG_PAD   # seg cols padded to no-match
    out[:a.shape[0]] = a
    return out


def _pad_rows(a, S):
    out = np.full((S,) + a.shape[1:], TRASH, a.dtype)
    out[:a.shape[0]] = a
    return out


def _emulate(pre, inputs):
    """Numpy emulation of the exact device dataflow (for validation)."""
    consts = pre["consts"]
    u1, u2pad, ucat = consts["u1"], consts["u2pad"], consts["ucat"]
    w1, b1 = consts["w1"], consts["b1"]
    w2pad, b2ppad = consts["w2pad"], consts["b2ppad"]
    SA, SB = pre["SA"], pre["SB"]
    x = np.asarray(inputs["x"], np.float32)

    def segsum(wseg_s, vals):
        """vals [128, T, C] -> slab [128, C] summed by seg id."""
        seg = wseg_s[:, 16:]
        segf = seg.reshape(-1).astype(np.int64)
        vf = vals.reshape(-1, vals.shape[-1])
        valid = segf < P
        slab = np.zeros((P, vals.shape[-1]), np.float32)
        np.add.at(slab, segf[valid], vf[valid])
        return slab

    D = []
    for c in pre["cores"]:
        Di = np.zeros((NROWS, 128), np.float32)
        for s in range(SA):
            w_ = c["wsegA"][s, :, :16]
            ex = np.exp(w_[:, :, None] * ucat[None, None, :])
            slab = segsum(c["wsegA"][s], ex)
            slab128 = np.zeros((P, 128), np.float32)
            slab128[:, :104] = slab
            rows = c["rowsA"][s]
            np.add.at(Di, rows, slab128)
        D.append(Di)

    P1full = np.zeros((N_NODES, 64), np.float32)
    for i in range(NCORES):
        h1 = x[i * NLOC:(i + 1) * NLOC] @ w1 + b1
        P1full[i * NLOC:(i + 1) * NLOC] = h1 / (D[i][:NLOC, :64] + EPS)

    def b_phase(core, table, u):
        gath = np.zeros((SB, P, T_B, 64), np.float32)
        for s in range(SB):
            for r in range(4):
                rows = core["gidxB"][s, r] * 4 + r            # [512]
                g = table[rows]                               # [512, 64]
                q = np.arange(CAP_CLASS)
                gath[s, q % P, 4 * r + q // P] = g
        slabs = []
        for s in range(SB):
            w_ = core["wsegB"][s, :, :16]
            ex = np.exp(w_[:, :, None] * u[None, None, :])
            msg = ex * gath[s]
            slabs.append(segsum(core["wsegB"][s], msg))
        return slabs

    P2full = np.zeros((N_NODES, 64), np.float32)
    for i, c in enumerate(pre["cores"]):
        slabs = b_phase(c, P1full, u1)
        P2l = np.zeros((NROWS + 1, 64), np.float32)
        for s in range(SB):
            o1 = slabs[s]
            q_ = np.maximum(o1, 0) + np.exp(np.minimum(o1, 0))
            h2 = q_ @ w2pad + b2ppad
            rows = c["rowsB"][s]
            Drows = D[i][np.minimum(rows, NROWS - 1), 64:128]
            p2 = h2 * (1.0 / (Drows + EPS))
            np.add.at(P2l, rows, p2)
        P2full[i * NLOC:(i + 1) * NLOC] = P2l[:NLOC]

    out = np.zeros((N_NODES, N_CLS), np.float32)
    for i, c in enumerate(pre["cores"]):
        slabs = b_phase(c, P2full, u2pad)
        OUT = np.zeros((NROWS + 1, 64), np.float32)
        for s in range(SB):
            o2 = slabs[s][:, :N_CLS]
            m = o2.max(axis=1, keepdims=True)
            e = np.exp(o2 - m)
            ls = (o2 - m) - np.log(e.sum(axis=1, keepdims=True))
            slab = np.zeros((P, 64), np.float32)
            slab[:, :N_CLS] = ls
            np.add.at(OUT, c["rowsB"][s], slab)
        out[i * NLOC:(i + 1) * NLOC] = OUT[:NLOC, :N_CLS]
    return out


# ---------------------------------------------------------------------------
# device program
# ---------------------------------------------------------------------------

def _build_program(pre, debug=False):
    import concourse.bacc as bacc
    import concourse.mybir as mybir
    import concourse.tile as tile

    SA, SB = pre["SA"], pre["SB"]
    GA8, GB8, G2 = SA // G8, SB // G8, SB // 2
    f32 = mybir.dt.float32
    i16 = mybir.dt.int16
    Exp = mybir.ActivationFunctionType.Exp
    Ln = mybir.ActivationFunctionType.Ln
    Relu = mybir.ActivationFunctionType.Relu
    X = mybir.AxisListType.X
    EQ = mybir.AluOpType.is_equal
    SUB = mybir.AluOpType.subtract

    nc = bacc.Bacc("TRN2", target_bir_lowering=False, debug=False,
                   num_devices=NCORES)

    xpre_d = nc.declare_dram_parameter("xpre", [NT_H1, 4, 125, P], f32, isOutput=False)
    wsegA_d = nc.declare_dram_parameter("wsegA", [SA, P, 32], f32, isOutput=False)
    rowsA_d = nc.declare_dram_parameter("rowsA16", [GA8, P, 64], i16, isOutput=False)
    wsegB_d = nc.declare_dram_parameter("wsegB", [SB, P, 32], f32, isOutput=False)
    rowsB_d = nc.declare_dram_parameter("rowsB16", [GB8, P, 64], i16, isOutput=False)
    gidx_d = nc.declare_dram_parameter("gidx16", [G2, P, 256], i16, isOutput=False)
    consts_d = nc.declare_dram_parameter("consts", [P, 680], f32, isOutput=False)
    w1_d = nc.declare_dram_parameter("w1c", [4, 125, 64], f32, isOutput=False)
    out_d = nc.declare_dram_parameter("out", [NROWS, 64], f32, isOutput=True)
    dbg = {}
    if debug:
        dbg["D"] = nc.declare_dram_parameter("dbg_D", [NROWS, 128], f32, isOutput=True)
        dbg["P1"] = nc.declare_dram_parameter("dbg_P1", [N_NODES, 64], f32, isOutput=True)
        dbg["P2"] = nc.declare_dram_parameter("dbg_P2", [N_NODES, 64], f32, isOutput=True)

    with tile.TileContext(nc) as tc:
        with (
            tc.tile_pool(name="cpool", bufs=1) as cpool,
            tc.tile_pool(name="dram", bufs=1, space="DRAM") as dpool,
            tc.tile_pool(name="big", bufs=2) as bpool,
            tc.tile_pool(name="small", bufs=3) as spool,
            tc.tile_pool(name="stage", bufs=2) as stpool,
            tc.tile_pool(name="psum", bufs=2, space="PSUM") as pp,
        ):
            # ---- DRAM internals ----
            D_t = dpool.tile([NROWS, 128], f32, tag="D")
            P1loc = dpool.tile([NROWS, 64], f32, tag="P1loc")
            P1full = dpool.tile([N_NODES, 64], f32, tag="P1full")
            P2loc = dpool.tile([NROWS, 64], f32, tag="P2loc")
            P2full = dpool.tile([N_NODES, 64], f32, tag="P2full")

            # ---- constants ----
            consts = cpool.tile([P, 680], f32, tag="consts")
            nc.sync.dma_start(out=consts[:], in_=consts_d[:])
            iota_t = consts[:, 0:128]
            u1b = consts[:, 128:192]
            u2b = consts[:, 192:256]
            ucatb = consts[:, 256:360]
            b1b = consts[:, 360:424]
            b2pb = consts[:, 424:488]
            w2sb = consts[0:64, 488:552]
            ident = consts[:, 552:680]
            w1sb = cpool.tile([125, 256], f32, tag="w1sb")
            for c in range(4):
                nc.sync.dma_start(out=w1sb[:, c * 64:(c + 1) * 64], in_=w1_d[c])
            zt = cpool.tile([P, 128], f32, tag="zt")
            nc.vector.memset(zt[:], 0.0)

            # ---- zero D, P2loc, out ----
            for a in range(NROWS // P):
                nc.sync.dma_start(
                    out=D_t[:].rearrange("(a p) c -> a p c", p=P)[a], in_=zt[:])
                nc.sync.dma_start(
                    out=P2loc[:].rearrange("(a p) c -> a p c", p=P)[a],
                    in_=zt[:, 0:64])
                nc.sync.dma_start(
                    out=out_d[:].rearrange("(a p) c -> a p c", p=P)[a],
                    in_=zt[:, 0:64])

            # ---- phase A: denominators ----
            for s in range(SA):
                meta = spool.tile([P, 32], f32, tag="metaA")
                nc.sync.dma_start(out=meta[:], in_=wsegA_d[s])
                ex = bpool.tile([P, T_A * 104], f32, tag="exA")
                for k in range(T_A):
                    nc.scalar.activation(
                        out=ex[:, k * 104:(k + 1) * 104], in_=ucatb,
                        func=Exp, scale=meta[:, k:k + 1])
                oh = bpool.tile([P, T_A * P], f32, tag="oh")
                nc.vector.tensor_tensor(
                    out=oh[:].rearrange("p (t q) -> p t q", t=T_A),
                    in0=meta[:, 16:32, None].to_broadcast([P, T_A, P]),
                    in1=iota_t[:, None, :].to_broadcast([P, T_A, P]), op=EQ)
                ps = pp.tile([P, 104], f32, space="PSUM", tag="ps")
                for k in range(T_A):
                    nc.tensor.matmul(
                        out=ps[:], lhsT=oh[:, k * P:(k + 1) * P],
                        rhs=ex[:, k * 104:(k + 1) * 104],
                        start=(k == 0), stop=(k == T_A - 1))
                s8 = s % G8
                if s8 == 0:
                    Aslabs = stpool.tile([P, G8 * 128], f32, tag="Aslabs")
                nc.vector.tensor_copy(
                    out=Aslabs[:, s8 * 128:s8 * 128 + 104], in_=ps[:])
                nc.vector.memset(Aslabs[:, s8 * 128 + 104:(s8 + 1) * 128], 0.0)
                if s8 == G8 - 1:
                    h = s // G8
                    ridx = spool.tile([P, 64], i16, tag="ridxA")
                    nc.sync.dma_start(out=ridx[:], in_=rowsA_d[h])
                    nc.gpsimd.dma_scatter_add(
                        out_ap=D_t[:],
                        in_ap=Aslabs[:].rearrange("p (j c) -> p j c", c=128),
                        idxs_ap=ridx[:], num_idxs=1024, num_idxs_reg=1024,
                        elem_size=128)

            # ---- h1 -> P1loc ----
            for j in range(NT_H1):
                xt = spool.tile([125, 512], f32, tag="xt")
                for c in range(4):
                    nc.sync.dma_start(
                        out=xt[:, c * 128:(c + 1) * 128], in_=xpre_d[j, c])
                hps = pp.tile([P, 64], f32, space="PSUM", tag="hps")
                for c in range(4):
                    nc.tensor.matmul(
                        out=hps[:], lhsT=xt[:, c * 128:(c + 1) * 128],
                        rhs=w1sb[:, c * 64:(c + 1) * 64],
                        start=(c == 0), stop=(c == 3))
                h1t = spool.tile([P, 64], f32, tag="h1t")
                nc.vector.tensor_add(out=h1t[:], in0=hps[:], in1=b1b)
                d1 = spool.tile([P, 64], f32, tag="d1")
                nc.sync.dma_start(
                    out=d1[:],
                    in_=D_t[:].rearrange("(a p) c -> a p c", p=P)[j][:, 0:64])
                nc.vector.tensor_scalar_add(d1[:], d1[:], EPS)
                rc = spool.tile([P, 64], f32, tag="rc")
                nc.vector.reciprocal(out=rc[:], in_=d1[:])
                p1t = spool.tile([P, 64], f32, tag="p1t")
                nc.vector.tensor_mul(out=p1t[:], in0=h1t[:], in1=rc[:])
                nc.sync.dma_start(
                    out=P1loc[:].rearrange("(a p) c -> a p c", p=P)[j],
                    in_=p1t[:])

            # ---- AllGather P1 ----
            nc.gpsimd.collective_compute(
                "AllGather", mybir.AluOpType.bypass,
                replica_groups=[list(range(NCORES))],
                ins=[P1loc[0:NLOC].opt()], outs=[P1full.opt()])

            # ---- phase B (shared) ----
            def b_phase(table, u_ap, layer):
                for s in range(SB):
                    g2, half = s // 2, s % 2
                    if half == 0:
                        gpair = bpool.tile([P, 32 * 64], f32, tag="gpair")
                        gix = spool.tile([P, 256], i16, tag="gix")
                        nc.sync.dma_start(out=gix[:], in_=gidx_d[g2])
                        tview = table[:].rearrange("(q f) c -> q (f c)", f=4)
                        for r in range(4):
                            nc.gpsimd.dma_gather(
                                out_ap=gpair[:, r * 512:(r + 1) * 512]
                                    .rearrange("p (t c) -> p t c", c=64),
                                in_ap=tview[:, r * 64:(r + 1) * 64],
                                idxs_ap=gix[:, r * 64:(r + 1) * 64],
                                num_idxs=1024, num_idxs_reg=1024,
                                elem_size=64, elem_step=256)
                    meta = spool.tile([P, 32], f32, tag="metaB")
                    nc.sync.dma_start(out=meta[:], in_=wsegB_d[s])
                    ex = bpool.tile([P, 1024], f32, tag="exB")
                    for k in range(T_B):
                        nc.scalar.activation(
                            out=ex[:, k * 64:(k + 1) * 64], in_=u_ap,
                            func=Exp, scale=meta[:, k:k + 1])
                    msg = bpool.tile([P, 1024], f32, tag="msg")
                    gv = gpair[:].rearrange("p (r h c) -> p r h c", r=4, h=2)[:, :, half, :]
                    nc.vector.tensor_tensor(
                        out=msg[:].rearrange("p (r c) -> p r c", r=4),
                        in0=ex[:].rearrange("p (r c) -> p r c", r=4),
                        in1=gv, op=mybir.AluOpType.mult)
                    oh = bpool.tile([P, T_B * P], f32, tag="oh")
                    nc.vector.tensor_tensor(
                        out=oh[:].rearrange("p (t q) -> p t q", t=T_B),
                        in0=meta[:, 16:32, None].to_broadcast([P, T_B, P]),
                        in1=iota_t[:, None, :].to_broadcast([P, T_B, P]), op=EQ)
                    ps = pp.tile([P, 104], f32, space="PSUM", tag="ps")
                    for k in range(T_B):
                        nc.tensor.matmul(
                            out=ps[:, 0:64], lhsT=oh[:, k * P:(k + 1) * P],
                            rhs=msg[:, k * 64:(k + 1) * 64],
                            start=(k == 0), stop=(k == T_B - 1))
                    s8 = s % G8
                    if s8 == 0:
                        Bslabs = stpool.tile([P, G8 * 64], f32, tag="Bslabs")
                    nc.vector.tensor_copy(
                        out=Bslabs[:, s8 * 64:(s8 + 1) * 64], in_=ps[:, 0:64])
                    if s8 == G8 - 1:
                        h = s // G8
                        ridx = spool.tile([P, 64], i16, tag="ridxB")
                        nc.sync.dma_start(out=ridx[:], in_=rowsB_d[h])
                        if layer == 1:
                            b1_epilogue(Bslabs, ridx)
                        else:
                            b2_epilogue(Bslabs, ridx)

            def b1_epilogue(Bslabs, ridx):
                d2g = bpool.tile([P, G8 * 128], f32, tag="d2g")
                nc.gpsimd.dma_gather(
                    out_ap=d2g[:].rearrange("p (j c) -> p j c", c=128),
                    in_ap=D_t[:], idxs_ap=ridx[:], num_idxs=1024,
                    num_idxs_reg=1024, elem_size=128)
                rel = spool.tile([P, 512], f32, tag="rel")
                nc.scalar.activation(out=rel[:], in_=Bslabs[:], func=Relu)
                mn = spool.tile([P, 512], f32, tag="mn")
                nc.vector.tensor_scalar_min(mn[:], Bslabs[:], 0.0)
                exm = spool.tile([P, 512], f32, tag="exm")
                nc.scalar.activation(out=exm[:], in_=mn[:], func=Exp)
                qq = spool.tile([P, 512], f32, tag="qq")
                nc.vector.tensor_add(out=qq[:], in0=rel[:], in1=exm[:])
                h2st = stpool.tile([P, 512], f32, tag="h2st")
                for j in range(G8):
                    tps = pp.tile([64, 128], f32, space="PSUM", tag="tps")
                    nc.tensor.transpose(
                        out=tps[:], in_=qq[:, j * 64:(j + 1) * 64],
                        identity=ident)
                    qT = spool.tile([64, 128], f32, tag="qT")
                    nc.vector.tensor_copy(out=qT[:], in_=tps[:])
                    h2ps = pp.tile([P, 64], f32, space="PSUM", tag="h2ps")
                    nc.tensor.matmul(out=h2ps[:], lhsT=qT[:], rhs=w2sb,
                                     start=True, stop=True)
                    nc.vector.tensor_copy(
                        out=h2st[:, j * 64:(j + 1) * 64], in_=h2ps[:])
                den8 = spool.tile([P, 512], f32, tag="den8")
                nc.vector.tensor_scalar_add(
                    den8[:].rearrange("p (j c) -> p j c", c=64),
                    d2g[:].rearrange("p (j c) -> p j c", c=128)[:, :, 64:128],
                    EPS)
                rec8 = spool.tile([P, 512], f32, tag="rec8")
                nc.vector.reciprocal(out=rec8[:], in_=den8[:])
                h2b = spool.tile([P, 512], f32, tag="h2b")
                nc.vector.tensor_add(
                    out=h2b[:].rearrange("p (j c) -> p j c", c=64),
                    in0=h2st[:].rearrange("p (j c) -> p j c", c=64),
                    in1=b2pb[:, None, :].to_broadcast([P, G8, 64]))
                p2st = stpool.tile([P, 512], f32, tag="p2st")
                nc.vector.tensor_mul(out=p2st[:], in0=h2b[:], in1=rec8[:])
                nc.gpsimd.dma_scatter_add(
                    out_ap=P2loc[:],
                    in_ap=p2st[:].rearrange("p (j c) -> p j c", c=64),
                    idxs_ap=ridx[:], num_idxs=1024, num_idxs_reg=1024,
                    elem_size=64)

            def b2_epilogue(Bslabs, ridx):
                Bv = Bslabs[:].rearrange("p (j c) -> p j c", c=64)[:, :, 0:40]
                mx8 = spool.tile([P, 8], f32, tag="mx8")
                nc.vector.reduce_max(mx8[:], Bv, axis=X)
                xm8 = spool.tile([P, 320], f32, tag="xm8")
                nc.vector.tensor_tensor(
                    out=xm8[:].rearrange("p (j c) -> p j c", c=40),
                    in0=Bv, in1=mx8[:, :, None].to_broadcast([P, G8, 40]),
                    op=SUB)
                ex8 = spool.tile([P, 320], f32, tag="ex8")
                nc.scalar.activation(out=ex8[:], in_=xm8[:], func=Exp)
                sm8 = spool.tile([P, 8], f32, tag="sm8")
                nc.vector.reduce_sum(
                    sm8[:], ex8[:].rearrange("p (j c) -> p j c", c=40), axis=X)
                ln8 = spool.tile([P, 8], f32, tag="ln8")
                nc.scalar.activation(out=ln8[:], in_=sm8[:], func=Ln)
                ost = stpool.tile([P, 512], f32, tag="ost")
                nc.vector.memset(ost[:], 0.0)
                nc.vector.tensor_tensor(
                    out=ost[:].rearrange("p (j c) -> p j c", c=64)[:, :, 0:40],
                    in0=xm8[:].rearrange("p (j c) -> p j c", c=40),
                    in1=ln8[:, :, None].to_broadcast([P, G8, 40]), op=SUB)
                nc.gpsimd.dma_scatter_add(
                    out_ap=out_d[:],
                    in_ap=ost[:].rearrange("p (j c) -> p j c", c=64),
                    idxs_ap=ridx[:], num_idxs=1024, num_idxs_reg=1024,
                    elem_size=64)

            b_phase(P1full, u1b, layer=1)

            # ---- AllGather P2 ----
            nc.gpsimd.collective_compute(
                "AllGather", mybir.AluOpType.bypass,
                replica_groups=[list(range(NCORES))],
                ins=[P2loc[0:NLOC].opt()], outs=[P2full.opt()])

            b_phase(P2full, u2b, layer=2)

            if debug:
                nc.sync.dma_start(out=dbg["D"][:], in_=D_t[:])
                nc.sync.dma_start(out=dbg["P1"][:], in_=P1full[:])
                nc.sync.dma_start(out=dbg["P2"][:], in_=P2full[:])

    nc.compile()
    return nc


def _make_consts_array(pre):
    c = pre["consts"]
    arr = np.zeros((P, 680), np.float32)
    arr[:, 0:128] = np.arange(128, dtype=np.float32)[None, :]
    arr[:, 128:192] = c["u1"][None, :]
    arr[:, 192:256] = c["u2pad"][None, :]
    arr[:, 256:360] = c["ucat"][None, :]
    arr[:, 360:424] = c["b1"][None, :]
    arr[:, 424:488] = c["b2ppad"][None, :]
    arr[0:64, 488:552] = c["w2pad"]
    arr[:, 552:680] = np.eye(128, dtype=np.float32)
    return arr


def _in_maps(pre):
    carr = _make_consts_array(pre)
    w1c = pre["consts"]["w1"].astype(np.float32)            # [500, 64]
    w1c = w1c.reshape(4, 125, 64).copy()
    maps = []
    for core in pre["cores"]:
        maps.append({
            "xpre": core["xpre"],
            "wsegA": core["wsegA"],
            "rowsA16": core["rowsA16"],
            "wsegB": core["wsegB"],
            "rowsB16": core["rowsB16"],
            "gidx16": core["gidx16"],
            "consts": carr,
            "w1c": w1c,
        })
    return maps


def _install_ntff_hook():
    """Register the axon NTFF profiling hook (missing antenv.axon_hooks in
    this image). Best effort — profiling only."""
    import sys, types
    try:
        import antenv  # noqa: F401
        if "antenv.axon_hooks" not in sys.modules:
            mod = types.ModuleType("antenv.axon_hooks")
            holder = [None]
            mod.set_axon_ntff_profile_hook = lambda h: holder.__setitem__(0, h)
            mod.get_axon_ntff_profile_hook = lambda: holder[0]
            sys.modules["antenv.axon_hooks"] = mod
            from trn_agent_boot.trn_boot import _ntff_profile_via_ctypes
            mod.set_axon_ntff_profile_hook(
                _ntff_profile_via_ctypes("/opt/axon/libaxon_pjrt.so"))
    except Exception:
        pass


def _run(inputs, profile=False, debug=False):
    from concourse.bass_utils import run_bass_kernel_spmd
    if profile:
        _install_ntff_hook()
    pre = _preprocess(inputs)
    nc = _build_program(pre, debug=debug)
    maps = _in_maps(pre)
    res = run_bass_kernel_spmd(nc, maps, list(range(NCORES)), trace=profile)
    out = np.concatenate(
        [res.results[i]["out"][:NLOC, :N_CLS] for i in range(NCORES)], axis=0)
    return out.astype(np.float32), res


def kernel(**inputs):
    out, _ = _run(inputs)
    return out



# revision 6
# speedup vs baseline: 1.8519x; 1.8519x over previous
"""Bass/Trainium2 kernel for nn_Net_40063454937541 (CurvGN 2-layer GNN).

Strategy (8 NeuronCores, SPMD single program):
  - Node space [100000] split into 8 contiguous ranges of 12500.
  - Exploits w_mul >= 0: leaky_relu(w_mul @ ma) is exactly linear in w_mul, so
    pre-softmax edge logits are affine in w_mul and the bias cancels in the
    segment softmax: softmax weights = exp(w_e * u[c]) / sum_src exp(w * u[c]).
  - Per-core node RELABELING: local nodes are bin-packed (first-fit) into SB
    uniform bins of 128 rows subject to per-bin caps on phase-A edges (by src),
    per-class phase-B edges (by dst), and a mod-4 position constraint
    (row % 4 == node % 4) preserving the int16 gather class trick. All
    supertiles therefore cover static row ranges [128*s, 128*(s+1)) identical
    on every core, so segment-sum slabs / D-row loads are plain HWDGE
    dma_starts instead of gpsimd scatter/gather descriptors. Host applies the
    inverse permutation to the output.
  - Phase A (edges by src): per-source denominators D[104] for both layers via
    one-hot matmul segment sums, written directly to static D rows.
  - h1 phase overlaps phase A: h1 = x @ w1 + b1 into SBUF; after A completes,
    P1 = h1/(D1+eps) -> P1loc; AllGather -> replicated P1 table.
  - Phase B (edges by dst): dma_gather P1[src] (mod-4 stride trick, int16
    indices, 4 SWDGE queues), msg = exp(w*u1)*P1[src], one-hot matmul segment
    sum by dst; fused epilogue ELU -> @w2 -> /D2 -> P2; AllGather P2; second
    pass with u2 -> log_softmax -> output rows.
All floating point math runs on device; host only shards/packs indices and
folds the tiny (64-wide) weight MLPs.
"""

import numpy as np

N_NODES = 100000
N_EDGES = 1600000
N_FEAT = 500
HID = 64
N_CLS = 40

NCORES = 8
NLOC = N_NODES // NCORES          # 12500
P = 128
T_B = 16                          # edge tiles per B supertile (4 classes x 4)
CLS_TILES = 4                     # tiles per mod-4 class region
CAP_CLASS = CLS_TILES * P         # 512 edge slots per class region
T_A = 16                          # edge tiles per A supertile
CAP_A = T_A * P                   # 2048
G8 = 4                            # supertiles per epilogue batch (SB % G8 == 0)
SEG_PAD = 200.0                   # one-hot never matches
EPS = 1e-16


def _wrap16(vals, dtype=np.int16):
    """Layout an index vector [n] (n % 16 == 0) into the q7 wrapped form
    [128, n//16]: position i lives at [i % 16, i // 16], replicated in all
    8 groups of 16 partitions."""
    v = np.asarray(vals)
    n = v.shape[0]
    assert n % 16 == 0
    w = v.reshape(n // 16, 16).T.astype(dtype)     # [16, n//16]
    return np.tile(w, (8, 1))                      # [128, n//16]


def _pack_bins(a_cnt, b_cnt, n_bins):
    """Worst-fit (load-balancing) node packing into n_bins uniform bins of
    128 positions. Constraints per bin: sum a_cnt <= CAP_A; per class r:
    sum b_cnt[r] <= CAP_CLASS; <= 32 nodes per residue class (position p
    holds only nodes with node % 4 == p % 4). Returns perm[n] -> row
    (bin*128 + pos) or None if infeasible."""
    n = len(a_cnt)
    order = np.argsort(-(a_cnt / CAP_A +
                         (b_cnt / CAP_CLASS).sum(axis=0)), kind="stable")
    a_used = np.zeros(n_bins)
    b_used = np.zeros((4, n_bins))
    res_used = np.zeros((4, n_bins), dtype=np.int64)
    perm = np.full(n, -1, dtype=np.int64)
    for node in order:
        r = node % 4
        na = a_used + a_cnt[node]
        nb = b_used + b_cnt[:, node][:, None]
        nr = res_used[r] + 1
        ok = (nr <= 32) & (na <= CAP_A) & np.all(nb <= CAP_CLASS, axis=0)
        if not ok.any():
            return None
        score = np.maximum((nb / CAP_CLASS).max(axis=0),
                           np.maximum(na / CAP_A, nr / 32.0))
        score[~ok] = 9e9
        s = int(np.argmin(score))
        a_used[s] = na[s]
        b_used[:, s] = nb[:, s]
        perm[node] = s * P + (res_used[r, s] * 4 + r)
        res_used[r, s] = nr[s]
    return perm


def _fold_weights(m1a, m1b_w, m1b_b, m2a, m2b_w, m2b_b, w2, b2):
    """Fold the tiny weight-MLPs using w_mul >= 0 (leaky_relu linear in w)."""
    s1 = np.where(m1a[0] >= 0, m1a[0], 0.2 * m1a[0])   # [64]
    u1 = (s1 @ m1b_w).astype(np.float32)               # [64]
    s2 = np.where(m2a[0] >= 0, m2a[0], 0.2 * m2a[0])   # [40]
    u2 = (s2 @ m2b_w).astype(np.float32)               # [40]
    # ELU fold: elu(x) = relu(x) + exp(min(x,0)) - 1; (q-1)@w2+b2 = q@w2+b2p
    b2p = (b2 - w2.sum(axis=0)).astype(np.float32)     # [40]
    return u1, u2, b2p


def _preprocess(inputs):
    """Build all per-core host arrays. Integer/layout work only (plus the
    tiny 64-wide weight folds)."""
    src = np.asarray(inputs["edge_index"][0], dtype=np.int64)
    dst = np.asarray(inputs["edge_index"][1], dtype=np.int64)
    w = np.asarray(inputs["w_mul"], dtype=np.float32).reshape(-1)
    x = np.asarray(inputs["x"], dtype=np.float32)

    u1, u2, b2p = _fold_weights(
        np.asarray(inputs["m1a"], np.float32), np.asarray(inputs["m1b_w"], np.float32),
        np.asarray(inputs["m1b_b"], np.float32), np.asarray(inputs["m2a"], np.float32),
        np.asarray(inputs["m2b_w"], np.float32), np.asarray(inputs["m2b_b"], np.float32),
        np.asarray(inputs["w2"], np.float32), np.asarray(inputs["b2"], np.float32))
    assert np.abs(u1).max() < 8 and np.abs(u2).max() < 8

    ucat = np.concatenate([u1, u2])                    # [104]
    u2pad = np.zeros(64, np.float32)
    u2pad[:N_CLS] = u2
    w2pad = np.zeros((64, 64), np.float32)
    w2pad[:, :N_CLS] = np.asarray(inputs["w2"], np.float32)
    b2ppad = np.zeros(64, np.float32)
    b2ppad[:N_CLS] = b2p

    core_shard = src // NLOC                           # phase A owner
    dst_shard = dst // NLOC                            # phase B owner
    cls = (src % 4).astype(np.int64)

    # per-core per-node counts
    a_cnts, b_cnts = [], []
    for i in range(NCORES):
        lo = i * NLOC
        m = core_shard == i
        a_cnts.append(np.bincount(src[m] - lo, minlength=NLOC))
        m = dst_shard == i
        b_cnts.append(np.stack([
            np.bincount(dst[m & (cls == r)] - lo, minlength=NLOC)
            for r in range(4)]))

    # find a bin count that packs every core
    SB = 96
    perms = None
    while perms is None:
        SB += 4
        assert SB <= 120, "node bin packing failed"
        trial = [_pack_bins(a_cnts[i], b_cnts[i], SB) for i in range(NCORES)]
        if all(p is not None for p in trial):
            perms = trial
    NROWS = SB * P

    # global table row of node n
    tab_row = np.empty(N_NODES, dtype=np.int64)
    for i in range(NCORES):
        tab_row[i * NLOC:(i + 1) * NLOC] = i * NROWS + perms[i]
    assert np.all((tab_row % 4) == (np.arange(N_NODES) % 4))

    cores = []
    for i in range(NCORES):
        lo = i * NLOC
        perm = perms[i]
        core = {"perm": perm}

        # ---------- phase A (by src, rows = perm[src-lo]) ----------
        m = core_shard == i
        ar, aw = perm[src[m] - lo], w[m]
        order = np.argsort(ar, kind="stable")
        ar, aw = ar[order], aw[order]
        wA = np.zeros((SB, P, T_A), np.float32)
        segA = np.full((SB, P, T_A), SEG_PAD, np.float32)
        sA = ar // P
        bnd = np.searchsorted(sA, np.arange(SB + 1))
        for s in range(SB):
            e0, e1 = bnd[s], bnd[s + 1]
            ne = e1 - e0
            assert ne <= CAP_A
            q = np.arange(ne)
            kk, pp = q // P, q % P
            wA[s, pp, kk] = aw[e0:e1]
            segA[s, pp, kk] = (ar[e0:e1] - s * P).astype(np.float32)
        core["wsegA"] = np.concatenate([wA, segA], axis=2)  # [SB,128,32]

        # ---------- phase B (by dst rows, classes by src%4) ----------
        m = dst_shard == i
        br, bg, bw, bc = perm[dst[m] - lo], src[m], w[m], cls[m]
        wB = np.zeros((SB, P, T_B), np.float32)
        segB = np.full((SB, P, T_B), SEG_PAD, np.float32)
        gidx = np.zeros((SB, 4, CAP_CLASS), np.int64)   # per supertile+class
        sB = br // P
        for r in range(4):
            mr = bc == r
            rr, rg, rw = br[mr], bg[mr], bw[mr]
            order = np.argsort(rr // P, kind="stable")
            rr, rg, rw = rr[order], rg[order], rw[order]
            bnd = np.searchsorted(rr // P, np.arange(SB + 1))
            for s in range(SB):
                a, b = bnd[s], bnd[s + 1]
                ne = b - a
                assert ne <= CAP_CLASS
                q = np.arange(ne)
                kk, pp = q // P, q % P
                wB[s, pp, 4 * r + kk] = rw[a:b]
                segB[s, pp, 4 * r + kk] = (rr[a:b] - s * P).astype(np.float32)
                gidx[s, r, :ne] = tab_row[rg[a:b]] // 4
        core["wsegB"] = np.concatenate([wB, segB], axis=2)  # [SB,128,32]
        core["gidxB"] = gidx

        # ---------- h1 phase: x pretransposed into permuted rows ----------
        xpad = np.zeros((NROWS, N_FEAT), np.float32)
        xpad[perm] = x[lo:lo + NLOC]
        xp = np.zeros((SB, 4, 125, P), np.float32)
        for j in range(SB):
            xp[j] = xpad[j * P:(j + 1) * P].T.reshape(4, 125, P)
        core["xpre"] = xp
        cores.append(core)

    # wrapped int16 gather indices, per pair of supertiles
    for c in cores:
        gi = c["gidxB"]                                    # [SB,4,512]
        pairs = gi.reshape(SB // 2, 2, 4, CAP_CLASS).transpose(0, 2, 1, 3)
        pairs = pairs.reshape(SB // 2, 4, 2 * CAP_CLASS)   # [G2,4,1024]
        c["gidx16"] = np.stack([
            np.concatenate([_wrap16(pairs[g, r]) for r in range(4)], axis=1)
            for g in range(SB // 2)])                      # [G2,128,256] i16

    consts = {
        "u1": u1, "u2pad": u2pad, "ucat": ucat, "w2pad": w2pad,
        "b2ppad": b2ppad,
        "b1": np.asarray(inputs["b1"], np.float32),
        "w1": np.asarray(inputs["w1"], np.float32),
    }
    return {"cores": cores, "SB": SB, "consts": consts}


def _emulate(pre, inputs):
    """Numpy emulation of the exact device dataflow (for validation)."""
    consts = pre["consts"]
    u1, u2pad, ucat = consts["u1"], consts["u2pad"], consts["ucat"]
    w1, b1 = consts["w1"], consts["b1"]
    w2pad, b2ppad = consts["w2pad"], consts["b2ppad"]
    SB = pre["SB"]
    NROWS = SB * P
    x = np.asarray(inputs["x"], np.float32)

    def segsum(wseg_s, vals):
        """vals [128, T, C] -> slab [128, C] summed by seg id."""
        seg = wseg_s[:, 16:]
        segf = seg.reshape(-1).astype(np.int64)
        vf = vals.reshape(-1, vals.shape[-1])
        valid = segf < P
        slab = np.zeros((P, vals.shape[-1]), np.float32)
        np.add.at(slab, segf[valid], vf[valid])
        return slab

    D = []
    for c in pre["cores"]:
        Di = np.zeros((NROWS, 128), np.float32)
        for s in range(SB):
            w_ = c["wsegA"][s, :, :16]
            ex = np.exp(w_[:, :, None] * ucat[None, None, :])
            slab = segsum(c["wsegA"][s], ex)
            Di[s * P:(s + 1) * P, :104] = slab
        D.append(Di)

    P1full = np.zeros((NCORES * NROWS, 64), np.float32)
    for i, c in enumerate(pre["cores"]):
        xpad = np.zeros((NROWS, N_FEAT), np.float32)
        xpad[c["perm"]] = x[i * NLOC:(i + 1) * NLOC]
        h1 = xpad @ w1 + b1
        P1full[i * NROWS:(i + 1) * NROWS] = h1 / (D[i][:, :64] + EPS)

    def b_phase(core, table, u):
        gath = np.zeros((SB, P, T_B, 64), np.float32)
        for s in range(SB):
            for r in range(4):
                rows = core["gidxB"][s, r] * 4 + r            # [512]
                g = table[rows]                               # [512, 64]
                q = np.arange(CAP_CLASS)
                gath[s, q % P, 4 * r + q // P] = g
        slabs = []
        for s in range(SB):
            w_ = core["wsegB"][s, :, :16]
            ex = np.exp(w_[:, :, None] * u[None, None, :])
            msg = ex * gath[s]
            slabs.append(segsum(core["wsegB"][s], msg))
        return slabs

    P2full = np.zeros((NCORES * NROWS, 64), np.float32)
    for i, c in enumerate(pre["cores"]):
        slabs = b_phase(c, P1full, u1)
        for s in range(SB):
            o1 = slabs[s]
            q_ = np.maximum(o1, 0) + np.exp(np.minimum(o1, 0))
            h2 = q_ @ w2pad + b2ppad
            p2 = h2 * (1.0 / (D[i][s * P:(s + 1) * P, 64:128] + EPS))
            P2full[i * NROWS + s * P:i * NROWS + (s + 1) * P] = p2

    out = np.zeros((N_NODES, N_CLS), np.float32)
    for i, c in enumerate(pre["cores"]):
        slabs = b_phase(c, P2full, u2pad)
        OUT = np.zeros((NROWS, 64), np.float32)
        for s in range(SB):
            o2 = slabs[s][:, :N_CLS]
            m = o2.max(axis=1, keepdims=True)
            e = np.exp(o2 - m)
            ls = (o2 - m) - np.log(e.sum(axis=1, keepdims=True))
            OUT[s * P:(s + 1) * P, :N_CLS] = ls
        out[i * NLOC:(i + 1) * NLOC] = OUT[c["perm"], :N_CLS]
    return out


# ---------------------------------------------------------------------------
# device program
# ---------------------------------------------------------------------------

def _build_program(pre, debug=False):
    import concourse.bacc as bacc
    import concourse.mybir as mybir
    import concourse.tile as tile

    SB = pre["SB"]
    NROWS = SB * P
    G2 = SB // 2
    f32 = mybir.dt.float32
    i16 = mybir.dt.int16
    Exp = mybir.ActivationFunctionType.Exp
    Ln = mybir.ActivationFunctionType.Ln
    Relu = mybir.ActivationFunctionType.Relu
    X = mybir.AxisListType.X
    EQ = mybir.AluOpType.is_equal
    SUB = mybir.AluOpType.subtract
    MUL = mybir.AluOpType.mult

    nc = bacc.Bacc("TRN2", target_bir_lowering=False, debug=False,
                   num_devices=NCORES, num_swdge_queues=4)

    xpre_d = nc.declare_dram_parameter("xpre", [SB, 4, 125, P], f32, isOutput=False)
    wsegA_d = nc.declare_dram_parameter("wsegA", [SB, P, 32], f32, isOutput=False)
    wsegB_d = nc.declare_dram_parameter("wsegB", [SB, P, 32], f32, isOutput=False)
    gidx_d = nc.declare_dram_parameter("gidx16", [G2, P, 256], i16, isOutput=False)
    consts_d = nc.declare_dram_parameter("consts", [P, 680], f32, isOutput=False)
    w1_d = nc.declare_dram_parameter("w1c", [4, 125, 64], f32, isOutput=False)
    out_d = nc.declare_dram_parameter("out", [NROWS, 64], f32, isOutput=True)
    dbg = {}
    if debug:
        dbg["D"] = nc.declare_dram_parameter("dbg_D", [NROWS, 128], f32, isOutput=True)
        dbg["P1"] = nc.declare_dram_parameter("dbg_P1", [NCORES * NROWS, 64], f32, isOutput=True)
        dbg["P2"] = nc.declare_dram_parameter("dbg_P2", [NCORES * NROWS, 64], f32, isOutput=True)

    with tile.TileContext(nc) as tc:
        with (
            tc.tile_pool(name="cpool", bufs=1) as cpool,
            tc.tile_pool(name="dram", bufs=1, space="DRAM") as dpool,
            tc.tile_pool(name="big", bufs=2) as bpool,
            tc.tile_pool(name="small", bufs=3) as spool,
            tc.tile_pool(name="stage", bufs=2) as stpool,
            tc.tile_pool(name="psum", bufs=2, space="PSUM") as pp,
        ):
            # ---- DRAM internals ----
            D_t = dpool.tile([NROWS, 128], f32, tag="D")
            P1loc = dpool.tile([NROWS, 64], f32, tag="P1loc")
            P1full = dpool.tile([NCORES * NROWS, 64], f32, tag="P1full")
            P2loc = dpool.tile([NROWS, 64], f32, tag="P2loc")
            P2full = dpool.tile([NCORES * NROWS, 64], f32, tag="P2full")

            D_rows = D_t[:].rearrange("(s p) c -> s p c", p=P)
            P1_rows = P1loc[:].rearrange("(s p) c -> s p c", p=P)
            P2_rows = P2loc[:].rearrange("(s p) c -> s p c", p=P)
            out_rows = out_d[:].rearrange("(s p) c -> s p c", p=P)

            # ---- constants ----
            consts = cpool.tile([P, 680], f32, tag="consts")
            nc.sync.dma_start(out=consts[:], in_=consts_d[:])
            iota_t = consts[:, 0:128]
            u1b = consts[:, 128:192]
            u2b = consts[:, 192:256]
            ucatb = consts[:, 256:360]
            b1b = consts[:, 360:424]
            b2pb = consts[:, 424:488]
            w2sb = consts[0:64, 488:552]
            ident = consts[:, 552:680]
            w1sb = cpool.tile([125, 256], f32, tag="w1sb")
            for c in range(4):
                nc.sync.dma_start(out=w1sb[:, c * 64:(c + 1) * 64], in_=w1_d[c])

            # h1 staging (SBUF-resident so the x@w1 matmuls overlap phase A)
            h1sb = cpool.tile([P, SB * 64], f32, tag="h1sb")

            # ---- phase A: denominators (static row writes) ----
            for s in range(SB):
                meta = spool.tile([P, 32], f32, tag="metaA")
                nc.sync.dma_start(out=meta[:], in_=wsegA_d[s])
                prodA = bpool.tile([P, T_A * 104], f32, tag="prodA")
                nc.vector.tensor_tensor(
                    out=prodA[:].rearrange("p (t c) -> p t c", t=T_A),
                    in0=meta[:, 0:16, None].to_broadcast([P, T_A, 104]),
                    in1=ucatb[:, None, :].to_broadcast([P, T_A, 104]), op=MUL)
                ex = bpool.tile([P, T_A * 104], f32, tag="exA")
                nc.scalar.activation(out=ex[:], in_=prodA[:], func=Exp)
                oh = bpool.tile([P, T_A * P], f32, tag="oh")
                nc.vector.tensor_tensor(
                    out=oh[:].rearrange("p (t q) -> p t q", t=T_A),
                    in0=meta[:, 16:32, None].to_broadcast([P, T_A, P]),
                    in1=iota_t[:, None, :].to_broadcast([P, T_A, P]), op=EQ)
                ps = pp.tile([P, 104], f32, space="PSUM", tag="ps")
                for k in range(T_A):
                    nc.tensor.matmul(
                        out=ps[:], lhsT=oh[:, k * P:(k + 1) * P],
                        rhs=ex[:, k * 104:(k + 1) * 104],
                        start=(k == 0), stop=(k == T_A - 1))
                At = stpool.tile([P, 128], f32, tag="At")
                nc.vector.tensor_copy(out=At[:, 0:104], in_=ps[:])
                nc.vector.memset(At[:, 104:128], 0.0)
                nc.sync.dma_start(out=D_rows[s], in_=At[:])

            # ---- h1 matmuls (independent of phase A; Tile overlaps) ----
            for j in range(SB):
                xt = spool.tile([125, 512], f32, tag="xt")
                for c in range(4):
                    nc.sync.dma_start(
                        out=xt[:, c * 128:(c + 1) * 128], in_=xpre_d[j, c])
                hps = pp.tile([P, 64], f32, space="PSUM", tag="hps")
                for c in range(4):
                    nc.tensor.matmul(
                        out=hps[:], lhsT=xt[:, c * 128:(c + 1) * 128],
                        rhs=w1sb[:, c * 64:(c + 1) * 64],
                        start=(c == 0), stop=(c == 3))
                nc.vector.tensor_add(
                    out=h1sb[:, j * 64:(j + 1) * 64], in0=hps[:], in1=b1b)

            # ---- P1 = h1 / (D1 + eps) ----
            for j in range(SB):
                d1 = spool.tile([P, 64], f32, tag="d1")
                nc.sync.dma_start(out=d1[:], in_=D_rows[j][:, 0:64])
                nc.vector.tensor_scalar_add(d1[:], d1[:], EPS)
                rc = spool.tile([P, 64], f32, tag="rc")
                nc.vector.reciprocal(out=rc[:], in_=d1[:])
                p1t = spool.tile([P, 64], f32, tag="p1t")
                nc.vector.tensor_mul(
                    out=p1t[:], in0=h1sb[:, j * 64:(j + 1) * 64], in1=rc[:])
                nc.sync.dma_start(out=P1_rows[j], in_=p1t[:])

            # ---- AllGather P1 ----
            nc.gpsimd.collective_compute(
                "AllGather", mybir.AluOpType.bypass,
                replica_groups=[list(range(NCORES))],
                ins=[P1loc[:].opt()], outs=[P1full.opt()])

            # ---- phase B (shared) ----
            def b_phase(table, u_ap, layer):
                for s in range(SB):
                    g2, half = s // 2, s % 2
                    if half == 0:
                        gpair = bpool.tile([P, 32 * 64], f32, tag="gpair")
                        gix = spool.tile([P, 256], i16, tag="gix")
                        nc.sync.dma_start(out=gix[:], in_=gidx_d[g2])
                        tview = table[:].rearrange("(q f) c -> q (f c)", f=4)
                        for r in range(4):
                            nc.gpsimd.dma_gather(
                                out_ap=gpair[:, r * 512:(r + 1) * 512]
                                    .rearrange("p (t c) -> p t c", c=64),
                                in_ap=tview[:, r * 64:(r + 1) * 64],
                                idxs_ap=gix[:, r * 64:(r + 1) * 64],
                                num_idxs=1024, num_idxs_reg=1024,
                                elem_size=64, elem_step=256,
                                queue_num=r)
                    meta = spool.tile([P, 32], f32, tag="metaB")
                    nc.sync.dma_start(out=meta[:], in_=wsegB_d[s])
                    prodB = bpool.tile([P, 1024], f32, tag="prodB")
                    nc.vector.tensor_tensor(
                        out=prodB[:].rearrange("p (t c) -> p t c", t=T_B),
                        in0=meta[:, 0:16, None].to_broadcast([P, T_B, 64]),
                        in1=u_ap[:, None, :].to_broadcast([P, T_B, 64]), op=MUL)
                    ex = bpool.tile([P, 1024], f32, tag="exB")
                    nc.scalar.activation(out=ex[:], in_=prodB[:], func=Exp)
                    msg = bpool.tile([P, 1024], f32, tag="msg")
                    gv = gpair[:].rearrange("p (r h c) -> p r h c", r=4, h=2)[:, :, half, :]
                    nc.vector.tensor_tensor(
                        out=msg[:].rearrange("p (r c) -> p r c", r=4),
                        in0=ex[:].rearrange("p (r c) -> p r c", r=4),
                        in1=gv, op=MUL)
                    oh = bpool.tile([P, T_B * P], f32, tag="oh")
                    nc.vector.tensor_tensor(
                        out=oh[:].rearrange("p (t q) -> p t q", t=T_B),
                        in0=meta[:, 16:32, None].to_broadcast([P, T_B, P]),
                        in1=iota_t[:, None, :].to_broadcast([P, T_B, P]), op=EQ)
                    ps = pp.tile([P, 104], f32, space="PSUM", tag="ps")
                    for k in range(T_B):
                        nc.tensor.matmul(
                            out=ps[:, 0:64], lhsT=oh[:, k * P:(k + 1) * P],
                            rhs=msg[:, k * 64:(k + 1) * 64],
                            start=(k == 0), stop=(k == T_B - 1))
                    s8 = s % G8
                    if s8 == 0:
                        Bslabs = stpool.tile([P, G8 * 64], f32, tag="Bslabs")
                    nc.vector.tensor_copy(
                        out=Bslabs[:, s8 * 64:(s8 + 1) * 64], in_=ps[:, 0:64])
                    if s8 == G8 - 1:
                        h = s // G8
                        if layer == 1:
                            b1_epilogue(Bslabs, h)
                        else:
                            b2_epilogue(Bslabs, h)

            def b1_epilogue(Bslabs, h):
                d2g = bpool.tile([P, G8 * 64], f32, tag="d2g")
                for j in range(G8):
                    nc.sync.dma_start(
                        out=d2g[:, j * 64:(j + 1) * 64],
                        in_=D_rows[h * G8 + j][:, 64:128])
                rel = spool.tile([P, G8 * 64], f32, tag="rel")
                nc.scalar.activation(out=rel[:], in_=Bslabs[:], func=Relu)
                mn = spool.tile([P, G8 * 64], f32, tag="mn")
                nc.vector.tensor_scalar_min(mn[:], Bslabs[:], 0.0)
                exm = spool.tile([P, G8 * 64], f32, tag="exm")
                nc.scalar.activation(out=exm[:], in_=mn[:], func=Exp)
                qq = spool.tile([P, G8 * 64], f32, tag="qq")
                nc.vector.tensor_add(out=qq[:], in0=rel[:], in1=exm[:])
                h2st = stpool.tile([P, G8 * 64], f32, tag="h2st")
                for j in range(G8):
                    tps = pp.tile([64, 128], f32, space="PSUM", tag="tps")
                    nc.tensor.transpose(
                        out=tps[:], in_=qq[:, j * 64:(j + 1) * 64],
                        identity=ident)
                    qT = spool.tile([64, 128], f32, tag="qT")
                    nc.vector.tensor_copy(out=qT[:], in_=tps[:])
                    h2ps = pp.tile([P, 64], f32, space="PSUM", tag="h2ps")
                    nc.tensor.matmul(out=h2ps[:], lhsT=qT[:], rhs=w2sb,
                                     start=True, stop=True)
                    nc.vector.tensor_copy(
                        out=h2st[:, j * 64:(j + 1) * 64], in_=h2ps[:])
                den8 = spool.tile([P, G8 * 64], f32, tag="den8")
                nc.vector.tensor_scalar_add(den8[:], d2g[:], EPS)
                rec8 = spool.tile([P, G8 * 64], f32, tag="rec8")
                nc.vector.reciprocal(out=rec8[:], in_=den8[:])
                h2b = spool.tile([P, G8 * 64], f32, tag="h2b")
                nc.vector.tensor_add(
                    out=h2b[:].rearrange("p (j c) -> p j c", c=64),
                    in0=h2st[:].rearrange("p (j c) -> p j c", c=64),
                    in1=b2pb[:, None, :].to_broadcast([P, G8, 64]))
                p2st = stpool.tile([P, G8 * 64], f32, tag="p2st")
                nc.vector.tensor_mul(out=p2st[:], in0=h2b[:], in1=rec8[:])
                for j in range(G8):
                    nc.sync.dma_start(
                        out=P2_rows[h * G8 + j],
                        in_=p2st[:, j * 64:(j + 1) * 64])

            def b2_epilogue(Bslabs, h):
                Bv = Bslabs[:].rearrange("p (j c) -> p j c", c=64)[:, :, 0:40]
                mx8 = spool.tile([P, G8], f32, tag="mx8")
                nc.vector.reduce_max(mx8[:], Bv, axis=X)
                xm8 = spool.tile([P, G8 * 40], f32, tag="xm8")
                nc.vector.tensor_tensor(
                    out=xm8[:].rearrange("p (j c) -> p j c", c=40),
                    in0=Bv, in1=mx8[:, :, None].to_broadcast([P, G8, 40]),
                    op=SUB)
                ex8 = spool.tile([P, G8 * 40], f32, tag="ex8")
                nc.scalar.activation(out=ex8[:], in_=xm8[:], func=Exp)
                sm8 = spool.tile([P, G8], f32, tag="sm8")
                nc.vector.reduce_sum(
                    sm8[:], ex8[:].rearrange("p (j c) -> p j c", c=40), axis=X)
                ln8 = spool.tile([P, G8], f32, tag="ln8")
                nc.scalar.activation(out=ln8[:], in_=sm8[:], func=Ln)
                ost = stpool.tile([P, G8 * 64], f32, tag="ost")
                nc.vector.memset(ost[:], 0.0)
                nc.vector.tensor_tensor(
                    out=ost[:].rearrange("p (j c) -> p j c", c=64)[:, :, 0:40],
                    in0=xm8[:].rearrange("p (j c) -> p j c", c=40),
                    in1=ln8[:, :, None].to_broadcast([P, G8, 40]), op=SUB)
                for j in range(G8):
                    nc.sync.dma_start(
                        out=out_rows[h * G8 + j],
                        in_=ost[:, j * 64:(j + 1) * 64])

            b_phase(P1full, u1b, layer=1)

            # ---- AllGather P2 ----
            nc.gpsimd.collective_compute(
                "AllGather", mybir.AluOpType.bypass,
                replica_groups=[list(range(NCORES))],
                ins=[P2loc[:].opt()], outs=[P2full.opt()])

            b_phase(P2full, u2b, layer=2)

            if debug:
                nc.sync.dma_start(out=dbg["D"][:], in_=D_t[:])
                nc.sync.dma_start(out=dbg["P1"][:], in_=P1full[:])
                nc.sync.dma_start(out=dbg["P2"][:], in_=P2full[:])

    nc.compile()
    return nc


def _make_consts_array(pre):
    c = pre["consts"]
    arr = np.zeros((P, 680), np.float32)
    arr[:, 0:128] = np.arange(128, dtype=np.float32)[None, :]
    arr[:, 128:192] = c["u1"][None, :]
    arr[:, 192:256] = c["u2pad"][None, :]
    arr[:, 256:360] = c["ucat"][None, :]
    arr[:, 360:424] = c["b1"][None, :]
    arr[:, 424:488] = c["b2ppad"][None, :]
    arr[0:64, 488:552] = c["w2pad"]
    arr[:, 552:680] = np.eye(128, dtype=np.float32)
    return arr


def _in_maps(pre):
    carr = _make_consts_array(pre)
    w1c = pre["consts"]["w1"].astype(np.float32)            # [500, 64]
    w1c = w1c.reshape(4, 125, 64).copy()
    maps = []
    for core in pre["cores"]:
        maps.append({
            "xpre": core["xpre"],
            "wsegA": core["wsegA"],
            "wsegB": core["wsegB"],
            "gidx16": core["gidx16"],
            "consts": carr,
            "w1c": w1c,
        })
    return maps


def _install_ntff_hook():
    """Register the axon NTFF profiling hook (missing antenv.axon_hooks in
    this image). Best effort — profiling only."""
    import sys, types
    try:
        import antenv  # noqa: F401
        if "antenv.axon_hooks" not in sys.modules:
            mod = types.ModuleType("antenv.axon_hooks")
            holder = [None]
            mod.set_axon_ntff_profile_hook = lambda h: holder.__setitem__(0, h)
            mod.get_axon_ntff_profile_hook = lambda: holder[0]
            sys.modules["antenv.axon_hooks"] = mod
            from trn_agent_boot.trn_boot import _ntff_profile_via_ctypes
            mod.set_axon_ntff_profile_hook(
                _ntff_profile_via_ctypes("/opt/axon/libaxon_pjrt.so"))
    except Exception:
        pass


def _run(inputs, profile=False, debug=False):
    from concourse.bass_utils import run_bass_kernel_spmd
    if profile:
        _install_ntff_hook()
    pre = _preprocess(inputs)
    nc = _build_program(pre, debug=debug)
    maps = _in_maps(pre)
    res = run_bass_kernel_spmd(nc, maps, list(range(NCORES)), trace=profile)
    out = np.concatenate(
        [res.results[i]["out"][pre["cores"][i]["perm"], :N_CLS]
         for i in range(NCORES)], axis=0)
    return out.astype(np.float32), res


def kernel(**inputs):
    out, _ = _run(inputs)
    return out


# revision 10
# speedup vs baseline: 2.1040x; 1.1361x over previous
"""Bass/Trainium2 kernel for nn_Net_40063454937541 (CurvGN 2-layer GNN).

Strategy (8 NeuronCores, SPMD single program):
  - Node space [100000] split into 8 contiguous ranges of 12500.
  - Exploits w_mul >= 0: leaky_relu(w_mul @ ma) is exactly linear in w_mul, so
    pre-softmax edge logits are affine in w_mul and the bias cancels in the
    segment softmax: softmax weights = exp(w_e * u[c]) / sum_src exp(w * u[c]).
  - Per-core node RELABELING: local nodes are bin-packed (first-fit) into SB
    uniform bins of 128 rows subject to per-bin caps on phase-A edges (by src),
    per-class phase-B edges (by dst), and a mod-4 position constraint
    (row % 4 == node % 4) preserving the int16 gather class trick. All
    supertiles therefore cover static row ranges [128*s, 128*(s+1)) identical
    on every core, so segment-sum slabs / D-row loads are plain HWDGE
    dma_starts instead of gpsimd scatter/gather descriptors. Host applies the
    inverse permutation to the output.
  - Phase A (edges by src): per-source denominators D[104] for both layers via
    one-hot matmul segment sums, written directly to static D rows.
  - h1 phase overlaps phase A: h1 = x @ w1 + b1 into SBUF; after A completes,
    P1 = h1/(D1+eps) -> P1loc; AllGather -> replicated P1 table.
  - Phase B (edges by dst): dma_gather P1[src] (mod-4 stride trick, int16
    indices, 4 SWDGE queues), msg = exp(w*u1)*P1[src], one-hot matmul segment
    sum by dst; fused epilogue ELU -> @w2 -> /D2 -> P2; AllGather P2; second
    pass with u2 -> log_softmax -> output rows.
All floating point math runs on device; host only shards/packs indices and
folds the tiny (64-wide) weight MLPs.
"""

import numpy as np

N_NODES = 100000
N_EDGES = 1600000
N_FEAT = 500
HID = 64
N_CLS = 40

NCORES = 8
NLOC = N_NODES // NCORES          # 12500
P = 128
T_B = 16                          # edge tiles per B supertile (4 classes x 4)
CLS_TILES = 4                     # tiles per mod-4 class region
CAP_CLASS = CLS_TILES * P         # 512 edge slots per class region
T_A = 16                          # edge tiles per A supertile
CAP_A = T_A * P                   # 2048
G8 = 4                            # supertiles per epilogue batch (SB % G8 == 0)
SEG_PAD = 200.0                   # one-hot never matches
EPS = 1e-16


def _wrap16(vals, dtype=np.int16):
    """Layout an index vector [n] (n % 16 == 0) into the q7 wrapped form
    [128, n//16]: position i lives at [i % 16, i // 16], replicated in all
    8 groups of 16 partitions."""
    v = np.asarray(vals)
    n = v.shape[0]
    assert n % 16 == 0
    w = v.reshape(n // 16, 16).T.astype(dtype)     # [16, n//16]
    return np.tile(w, (8, 1))                      # [128, n//16]


def _pack_bins(a_cnt, b_cnt, n_bins):
    """Worst-fit (load-balancing) node packing into n_bins uniform bins of
    128 positions. Constraints per bin: sum a_cnt <= CAP_A; per class r:
    sum b_cnt[r] <= CAP_CLASS; <= 32 nodes per residue class (position p
    holds only nodes with node % 4 == p % 4). Returns perm[n] -> row
    (bin*128 + pos) or None if infeasible."""
    n = len(a_cnt)
    order = np.argsort(-(a_cnt / CAP_A +
                         (b_cnt / CAP_CLASS).sum(axis=0)), kind="stable")
    a_used = np.zeros(n_bins)
    b_used = np.zeros((4, n_bins))
    res_used = np.zeros((4, n_bins), dtype=np.int64)
    perm = np.full(n, -1, dtype=np.int64)
    for node in order:
        r = node % 4
        na = a_used + a_cnt[node]
        nb = b_used + b_cnt[:, node][:, None]
        nr = res_used[r] + 1
        ok = (nr <= 32) & (na <= CAP_A) & np.all(nb <= CAP_CLASS, axis=0)
        if not ok.any():
            return None
        score = np.maximum((nb / CAP_CLASS).max(axis=0),
                           np.maximum(na / CAP_A, nr / 32.0))
        score[~ok] = 9e9
        s = int(np.argmin(score))
        a_used[s] = na[s]
        b_used[:, s] = nb[:, s]
        perm[node] = s * P + (res_used[r, s] * 4 + r)
        res_used[r, s] = nr[s]
    return perm


def _fold_weights(m1a, m1b_w, m1b_b, m2a, m2b_w, m2b_b, w2, b2):
    """Fold the tiny weight-MLPs using w_mul >= 0 (leaky_relu linear in w)."""
    s1 = np.where(m1a[0] >= 0, m1a[0], 0.2 * m1a[0])   # [64]
    u1 = (s1 @ m1b_w).astype(np.float32)               # [64]
    s2 = np.where(m2a[0] >= 0, m2a[0], 0.2 * m2a[0])   # [40]
    u2 = (s2 @ m2b_w).astype(np.float32)               # [40]
    # ELU fold: elu(x) = relu(x) + exp(min(x,0)) - 1; (q-1)@w2+b2 = q@w2+b2p
    b2p = (b2 - w2.sum(axis=0)).astype(np.float32)     # [40]
    return u1, u2, b2p


def _preprocess(inputs):
    """Build all per-core host arrays. Integer/layout work only (plus the
    tiny 64-wide weight folds)."""
    src = np.asarray(inputs["edge_index"][0], dtype=np.int64)
    dst = np.asarray(inputs["edge_index"][1], dtype=np.int64)
    w = np.asarray(inputs["w_mul"], dtype=np.float32).reshape(-1)
    x = np.asarray(inputs["x"], dtype=np.float32)

    u1, u2, b2p = _fold_weights(
        np.asarray(inputs["m1a"], np.float32), np.asarray(inputs["m1b_w"], np.float32),
        np.asarray(inputs["m1b_b"], np.float32), np.asarray(inputs["m2a"], np.float32),
        np.asarray(inputs["m2b_w"], np.float32), np.asarray(inputs["m2b_b"], np.float32),
        np.asarray(inputs["w2"], np.float32), np.asarray(inputs["b2"], np.float32))
    assert np.abs(u1).max() < 8 and np.abs(u2).max() < 8

    ucat = np.concatenate([u1, u2])                    # [104]
    u2pad = np.zeros(64, np.float32)
    u2pad[:N_CLS] = u2
    w2pad = np.zeros((64, 64), np.float32)
    w2pad[:, :N_CLS] = np.asarray(inputs["w2"], np.float32)
    b2ppad = np.zeros(64, np.float32)
    b2ppad[:N_CLS] = b2p

    core_shard = src // NLOC                           # phase A owner
    dst_shard = dst // NLOC                            # phase B owner
    cls = (src % 4).astype(np.int64)

    # per-core per-node counts
    a_cnts, b_cnts = [], []
    for i in range(NCORES):
        lo = i * NLOC
        m = core_shard == i
        a_cnts.append(np.bincount(src[m] - lo, minlength=NLOC))
        m = dst_shard == i
        b_cnts.append(np.stack([
            np.bincount(dst[m & (cls == r)] - lo, minlength=NLOC)
            for r in range(4)]))

    # find a bin count that packs every core
    SB = 80
    perms = None
    while perms is None:
        SB += 20              # SB % 20 == 0: 5 AllGather chunks of G8-aligned bins
        assert SB <= 140, "node bin packing failed"
        trial = [_pack_bins(a_cnts[i], b_cnts[i], SB) for i in range(NCORES)]
        if all(p is not None for p in trial):
            perms = trial
    NROWS = SB * P

    # global table row of node n — chunk-major layout so each AllGather
    # chunk writes one contiguous block: row = chunk*(8*CBP) + core*CBP + pos
    NCH = 5
    assert SB % NCH == 0 and (SB // NCH) % G8 == 0
    CBP = (SB // NCH) * P
    tab_row = np.empty(N_NODES, dtype=np.int64)
    for i in range(NCORES):
        pr = perms[i]
        tab_row[i * NLOC:(i + 1) * NLOC] = (
            (pr // CBP) * (NCORES * CBP) + i * CBP + (pr % CBP))
    assert np.all((tab_row % 4) == (np.arange(N_NODES) % 4))

    cores = []
    for i in range(NCORES):
        lo = i * NLOC
        perm = perms[i]
        core = {"perm": perm}

        # ---------- phase A (by src, rows = perm[src-lo]) ----------
        m = core_shard == i
        ar, aw = perm[src[m] - lo], w[m]
        order = np.argsort(ar, kind="stable")
        ar, aw = ar[order], aw[order]
        wA = np.zeros((SB, P, T_A), np.float32)
        segA = np.full((SB, P, T_A), SEG_PAD, np.float32)
        sA = ar // P
        bnd = np.searchsorted(sA, np.arange(SB + 1))
        for s in range(SB):
            e0, e1 = bnd[s], bnd[s + 1]
            ne = e1 - e0
            assert ne <= CAP_A
            q = np.arange(ne)
            kk, pp = q // P, q % P
            wA[s, pp, kk] = aw[e0:e1]
            segA[s, pp, kk] = (ar[e0:e1] - s * P).astype(np.float32)
        core["wsegA"] = np.concatenate([wA, segA], axis=2)  # [SB,128,32]

        # ---------- phase B (by dst rows, classes by src%4) ----------
        m = dst_shard == i
        br, bg, bw, bc = perm[dst[m] - lo], src[m], w[m], cls[m]
        wB = np.zeros((SB, P, T_B), np.float32)
        segB = np.full((SB, P, T_B), SEG_PAD, np.float32)
        gidx = np.zeros((SB, 4, CAP_CLASS), np.int64)   # per supertile+class
        sB = br // P
        for r in range(4):
            mr = bc == r
            rr, rg, rw = br[mr], bg[mr], bw[mr]
            order = np.argsort(rr // P, kind="stable")
            rr, rg, rw = rr[order], rg[order], rw[order]
            bnd = np.searchsorted(rr // P, np.arange(SB + 1))
            for s in range(SB):
                a, b = bnd[s], bnd[s + 1]
                ne = b - a
                assert ne <= CAP_CLASS
                q = np.arange(ne)
                kk, pp = q // P, q % P
                wB[s, pp, 4 * r + kk] = rw[a:b]
                segB[s, pp, 4 * r + kk] = (rr[a:b] - s * P).astype(np.float32)
                gidx[s, r, :ne] = tab_row[rg[a:b]] // 4
        core["wsegB"] = np.concatenate([wB, segB], axis=2)  # [SB,128,32]
        core["gidxB"] = gidx

        # ---------- h1 phase: x pretransposed into permuted rows ----------
        xpad = np.zeros((NROWS, N_FEAT), np.float32)
        xpad[perm] = x[lo:lo + NLOC]
        xp = np.zeros((SB, 4, 125, P), np.float32)
        for j in range(SB):
            xp[j] = xpad[j * P:(j + 1) * P].T.reshape(4, 125, P)
        core["xpre"] = xp
        cores.append(core)

    # wrapped int16 gather indices, per pair of supertiles
    for c in cores:
        gi = c["gidxB"]                                    # [SB,4,512]
        pairs = gi.reshape(SB // 2, 2, 4, CAP_CLASS).transpose(0, 2, 1, 3)
        pairs = pairs.reshape(SB // 2, 4, 2 * CAP_CLASS)   # [G2,4,1024]
        c["gidx16"] = np.stack([
            np.concatenate([_wrap16(pairs[g, r]) for r in range(4)], axis=1)
            for g in range(SB // 2)])                      # [G2,128,256] i16

    consts = {
        "u1": u1, "u2pad": u2pad, "ucat": ucat, "w2pad": w2pad,
        "b2ppad": b2ppad,
        "b1": np.asarray(inputs["b1"], np.float32),
        "w1": np.asarray(inputs["w1"], np.float32),
    }
    return {"cores": cores, "SB": SB, "tab_row": tab_row, "consts": consts}


def _emulate(pre, inputs):
    """Numpy emulation of the exact device dataflow (for validation)."""
    consts = pre["consts"]
    u1, u2pad, ucat = consts["u1"], consts["u2pad"], consts["ucat"]
    w1, b1 = consts["w1"], consts["b1"]
    w2pad, b2ppad = consts["w2pad"], consts["b2ppad"]
    SB = pre["SB"]
    NROWS = SB * P
    x = np.asarray(inputs["x"], np.float32)

    def segsum(wseg_s, vals):
        """vals [128, T, C] -> slab [128, C] summed by seg id."""
        seg = wseg_s[:, 16:]
        segf = seg.reshape(-1).astype(np.int64)
        vf = vals.reshape(-1, vals.shape[-1])
        valid = segf < P
        slab = np.zeros((P, vals.shape[-1]), np.float32)
        np.add.at(slab, segf[valid], vf[valid])
        return slab

    D = []
    for c in pre["cores"]:
        Di = np.zeros((NROWS, 128), np.float32)
        for s in range(SB):
            w_ = c["wsegA"][s, :, :16]
            ex = np.exp(w_[:, :, None] * ucat[None, None, :])
            slab = segsum(c["wsegA"][s], ex)
            Di[s * P:(s + 1) * P, :104] = slab
        D.append(Di)

    tab = pre["tab_row"]
    P1full = np.zeros((NCORES * NROWS, 64), np.float32)
    for i, c in enumerate(pre["cores"]):
        xpad = np.zeros((NROWS, N_FEAT), np.float32)
        xpad[c["perm"]] = x[i * NLOC:(i + 1) * NLOC]
        h1 = xpad @ w1 + b1
        P1loc = h1 / (D[i][:, :64] + EPS)
        P1full[tab[i * NLOC:(i + 1) * NLOC]] = P1loc[c["perm"]]

    def b_phase(core, table, u):
        gath = np.zeros((SB, P, T_B, 64), np.float32)
        for s in range(SB):
            for r in range(4):
                rows = core["gidxB"][s, r] * 4 + r            # [512]
                g = table[rows]                               # [512, 64]
                q = np.arange(CAP_CLASS)
                gath[s, q % P, 4 * r + q // P] = g
        slabs = []
        for s in range(SB):
            w_ = core["wsegB"][s, :, :16]
            ex = np.exp(w_[:, :, None] * u[None, None, :])
            msg = ex * gath[s]
            slabs.append(segsum(core["wsegB"][s], msg))
        return slabs

    P2full = np.zeros((NCORES * NROWS, 64), np.float32)
    for i, c in enumerate(pre["cores"]):
        slabs = b_phase(c, P1full, u1)
        P2loc = np.zeros((NROWS, 64), np.float32)
        for s in range(SB):
            o1 = slabs[s]
            q_ = np.maximum(o1, 0) + np.exp(np.minimum(o1, 0))
            h2 = q_ @ w2pad + b2ppad
            p2 = h2 * (1.0 / (D[i][s * P:(s + 1) * P, 64:128] + EPS))
            P2loc[s * P:(s + 1) * P] = p2
        P2full[tab[i * NLOC:(i + 1) * NLOC]] = P2loc[c["perm"]]

    out = np.zeros((N_NODES, N_CLS), np.float32)
    for i, c in enumerate(pre["cores"]):
        slabs = b_phase(c, P2full, u2pad)
        OUT = np.zeros((NROWS, 64), np.float32)
        for s in range(SB):
            o2 = slabs[s][:, :N_CLS]
            m = o2.max(axis=1, keepdims=True)
            e = np.exp(o2 - m)
            ls = (o2 - m) - np.log(e.sum(axis=1, keepdims=True))
            OUT[s * P:(s + 1) * P, :N_CLS] = ls
        out[i * NLOC:(i + 1) * NLOC] = OUT[c["perm"], :N_CLS]
    return out


# ---------------------------------------------------------------------------
# device program
# ---------------------------------------------------------------------------

def _build_program(pre, debug=False):
    import concourse.bacc as bacc
    import concourse.mybir as mybir
    import concourse.tile as tile

    SB = pre["SB"]
    NROWS = SB * P
    G2 = SB // 2
    NCH = 5                        # AllGather chunks
    assert SB % NCH == 0 and (SB // NCH) % G8 == 0
    CB = SB // NCH                 # bins per chunk
    f32 = mybir.dt.float32
    i16 = mybir.dt.int16
    Exp = mybir.ActivationFunctionType.Exp
    Ln = mybir.ActivationFunctionType.Ln
    Relu = mybir.ActivationFunctionType.Relu
    X = mybir.AxisListType.X
    EQ = mybir.AluOpType.is_equal
    SUB = mybir.AluOpType.subtract
    MUL = mybir.AluOpType.mult

    nc = bacc.Bacc("TRN2", target_bir_lowering=False, debug=False,
                   num_devices=NCORES, num_swdge_queues=4)

    xpre_d = nc.declare_dram_parameter("xpre", [SB, 4, 125, P], f32, isOutput=False)
    wsegA_d = nc.declare_dram_parameter("wsegA", [SB, P, 32], f32, isOutput=False)
    wsegB_d = nc.declare_dram_parameter("wsegB", [SB, P, 32], f32, isOutput=False)
    gidx_d = nc.declare_dram_parameter("gidx16", [G2, P, 256], i16, isOutput=False)
    consts_d = nc.declare_dram_parameter("consts", [P, 680], f32, isOutput=False)
    w1_d = nc.declare_dram_parameter("w1c", [4, 125, 64], f32, isOutput=False)
    out_d = nc.declare_dram_parameter("out", [NROWS, 64], f32, isOutput=True)
    dbg = {}
    if debug:
        dbg["D"] = nc.declare_dram_parameter("dbg_D", [NROWS, 128], f32, isOutput=True)
        dbg["P1"] = nc.declare_dram_parameter("dbg_P1", [NCORES * NROWS, 64], f32, isOutput=True)
        dbg["P2"] = nc.declare_dram_parameter("dbg_P2", [NCORES * NROWS, 64], f32, isOutput=True)

    with tile.TileContext(nc) as tc:
        with (
            tc.tile_pool(name="cpool", bufs=1) as cpool,
            tc.tile_pool(name="dram", bufs=1, space="DRAM") as dpool,
            tc.tile_pool(name="big", bufs=3) as bpool,
            tc.tile_pool(name="small", bufs=3) as spool,
            tc.tile_pool(name="stage", bufs=2) as stpool,
            tc.tile_pool(name="psum", bufs=2, space="PSUM") as pp,
        ):
            # ---- DRAM internals (per-chunk tiles so collective deps are
            # exact even under coarse whole-tile tracking) ----
            D_ts = [dpool.tile([CB * P, 128], f32, tag=f"D{c}", name=f"D{c}")
                    for c in range(NCH)]
            P1locs = [dpool.tile([CB * P, 64], f32, tag=f"P1loc{c}", name=f"P1loc{c}")
                      for c in range(NCH)]
            P2locs = [dpool.tile([CB * P, 64], f32, tag=f"P2loc{c}", name=f"P2loc{c}")
                      for c in range(NCH)]
            P1full = dpool.tile([NCORES * NROWS, 64], f32, tag="P1full")
            P2full = dpool.tile([NCORES * NROWS, 64], f32, tag="P2full")

            def D_row(j):
                return D_ts[j // CB][:].rearrange("(s p) c -> s p c", p=P)[j % CB]

            def P1_row(j):
                return P1locs[j // CB][:].rearrange("(s p) c -> s p c", p=P)[j % CB]

            def P2_row(j):
                return P2locs[j // CB][:].rearrange("(s p) c -> s p c", p=P)[j % CB]

            out_rows = out_d[:].rearrange("(s p) c -> s p c", p=P)

            # ---- constants ----
            consts = cpool.tile([P, 680], f32, tag="consts")
            nc.sync.dma_start(out=consts[:], in_=consts_d[:])
            iota_t = consts[:, 0:128]
            u1b = consts[:, 128:192]
            u2b = consts[:, 192:256]
            ucatb = consts[:, 256:360]
            b1b = consts[:, 360:424]
            b2pb = consts[:, 424:488]
            w2sb = consts[0:64, 488:552]
            ident = consts[:, 552:680]
            w1sb = cpool.tile([125, 256], f32, tag="w1sb")
            for c in range(4):
                nc.sync.dma_start(out=w1sb[:, c * 64:(c + 1) * 64], in_=w1_d[c])

            # h1 staging (SBUF-resident so the x@w1 matmuls overlap phase A)
            h1sb = cpool.tile([P, SB * 64], f32, tag="h1sb")

            # ---- phase A + h1 + P1 divide + chunked AllGather ----
            for ch in range(NCH):
                for s in range(ch * CB, (ch + 1) * CB):
                    meta = spool.tile([P, 32], f32, tag="metaA")
                    nc.sync.dma_start(out=meta[:], in_=wsegA_d[s])
                    prodA = bpool.tile([P, T_A * 104], f32, tag="prodA")
                    nc.vector.tensor_tensor(
                        out=prodA[:].rearrange("p (t c) -> p t c", t=T_A),
                        in0=meta[:, 0:16, None].to_broadcast([P, T_A, 104]),
                        in1=ucatb[:, None, :].to_broadcast([P, T_A, 104]), op=MUL)
                    ex = bpool.tile([P, T_A * 104], f32, tag="exA")
                    nc.scalar.activation(out=ex[:], in_=prodA[:], func=Exp)
                    oh = bpool.tile([P, T_A * P], f32, tag="oh")
                    nc.vector.tensor_tensor(
                        out=oh[:].rearrange("p (t q) -> p t q", t=T_A),
                        in0=meta[:, 16:32, None].to_broadcast([P, T_A, P]),
                        in1=iota_t[:, None, :].to_broadcast([P, T_A, P]), op=EQ)
                    ps = pp.tile([P, 104], f32, space="PSUM", tag="ps")
                    for k in range(T_A):
                        nc.tensor.matmul(
                            out=ps[:], lhsT=oh[:, k * P:(k + 1) * P],
                            rhs=ex[:, k * 104:(k + 1) * 104],
                            start=(k == 0), stop=(k == T_A - 1))
                    At = stpool.tile([P, 128], f32, tag="At")
                    nc.vector.tensor_copy(out=At[:, 0:104], in_=ps[:])
                    nc.vector.memset(At[:, 104:128], 0.0)
                    nc.sync.dma_start(out=D_row(s), in_=At[:])

                for j in range(ch * CB, (ch + 1) * CB):
                    xt = spool.tile([125, 512], f32, tag="xt")
                    nc.scalar.dma_start(
                        out=xt[:].rearrange("p (c f) -> p c f", c=4),
                        in_=xpre_d[j].rearrange("c p f -> p c f"))
                    hps = pp.tile([P, 64], f32, space="PSUM", tag="hps")
                    for c in range(4):
                        nc.tensor.matmul(
                            out=hps[:], lhsT=xt[:, c * 128:(c + 1) * 128],
                            rhs=w1sb[:, c * 64:(c + 1) * 64],
                            start=(c == 0), stop=(c == 3))
                    nc.vector.tensor_add(
                        out=h1sb[:, j * 64:(j + 1) * 64], in0=hps[:], in1=b1b)

                for j in range(ch * CB, (ch + 1) * CB):
                    d1 = spool.tile([P, 64], f32, tag="d1")
                    nc.scalar.dma_start(out=d1[:], in_=D_row(j)[:, 0:64])
                    nc.vector.tensor_scalar_add(d1[:], d1[:], EPS)
                    rc = spool.tile([P, 64], f32, tag="rc")
                    nc.vector.reciprocal(out=rc[:], in_=d1[:])
                    p1t = spool.tile([P, 64], f32, tag="p1t")
                    nc.vector.tensor_mul(
                        out=p1t[:], in0=h1sb[:, j * 64:(j + 1) * 64], in1=rc[:])
                    nc.sync.dma_start(out=P1_row(j), in_=p1t[:])

                nc.gpsimd.collective_compute(
                    "AllGather", mybir.AluOpType.bypass,
                    replica_groups=[list(range(NCORES))],
                    ins=[P1locs[ch][:].opt()],
                    outs=[P1full[ch * NCORES * CB * P:
                                 (ch + 1) * NCORES * CB * P].opt()])

            # ---- phase B (shared) ----
            def b_phase(table, u_ap, layer):
                for s in range(SB):
                    g2, half = s // 2, s % 2
                    if half == 0:
                        gpair = bpool.tile([P, 32 * 64], f32, tag="gpair")
                        gix = spool.tile([P, 256], i16, tag="gix")
                        nc.sync.dma_start(out=gix[:], in_=gidx_d[g2])
                        tview = table[:].rearrange("(q f) c -> q (f c)", f=4)
                        for r in range(4):
                            nc.gpsimd.dma_gather(
                                out_ap=gpair[:, r * 512:(r + 1) * 512]
                                    .rearrange("p (t c) -> p t c", c=64),
                                in_ap=tview[:, r * 64:(r + 1) * 64],
                                idxs_ap=gix[:, r * 64:(r + 1) * 64],
                                num_idxs=1024, num_idxs_reg=1024,
                                elem_size=64, elem_step=256,
                                queue_num=r)
                    meta = spool.tile([P, 32], f32, tag="metaB")
                    nc.sync.dma_start(out=meta[:], in_=wsegB_d[s])
                    prodB = bpool.tile([P, 1024], f32, tag="prodB")
                    nc.vector.tensor_tensor(
                        out=prodB[:].rearrange("p (t c) -> p t c", t=T_B),
                        in0=meta[:, 0:16, None].to_broadcast([P, T_B, 64]),
                        in1=u_ap[:, None, :].to_broadcast([P, T_B, 64]), op=MUL)
                    ex = bpool.tile([P, 1024], f32, tag="exB")
                    nc.scalar.activation(out=ex[:], in_=prodB[:], func=Exp)
                    msg = bpool.tile([P, 1024], f32, tag="msg")
                    gv = gpair[:].rearrange("p (r h c) -> p r h c", r=4, h=2)[:, :, half, :]
                    nc.vector.tensor_tensor(
                        out=msg[:].rearrange("p (r c) -> p r c", r=4),
                        in0=ex[:].rearrange("p (r c) -> p r c", r=4),
                        in1=gv, op=MUL)
                    oh = bpool.tile([P, T_B * P], f32, tag="oh")
                    nc.vector.tensor_tensor(
                        out=oh[:].rearrange("p (t q) -> p t q", t=T_B),
                        in0=meta[:, 16:32, None].to_broadcast([P, T_B, P]),
                        in1=iota_t[:, None, :].to_broadcast([P, T_B, P]), op=EQ)
                    ps = pp.tile([P, 104], f32, space="PSUM", tag="ps")
                    for k in range(T_B):
                        nc.tensor.matmul(
                            out=ps[:, 0:64], lhsT=oh[:, k * P:(k + 1) * P],
                            rhs=msg[:, k * 64:(k + 1) * 64],
                            start=(k == 0), stop=(k == T_B - 1))
                    s8 = s % G8
                    if s8 == 0:
                        Bslabs = stpool.tile([P, G8 * 64], f32, tag="Bslabs")
                    nc.vector.tensor_copy(
                        out=Bslabs[:, s8 * 64:(s8 + 1) * 64], in_=ps[:, 0:64])
                    if s8 == G8 - 1:
                        h = s // G8
                        if layer == 1:
                            b1_epilogue(Bslabs, h)
                            if (s + 1) % CB == 0:
                                ch = (s + 1) // CB - 1
                                nc.gpsimd.collective_compute(
                                    "AllGather", mybir.AluOpType.bypass,
                                    replica_groups=[list(range(NCORES))],
                                    ins=[P2locs[ch][:].opt()],
                                    outs=[P2full[ch * NCORES * CB * P:
                                                 (ch + 1) * NCORES * CB * P].opt()])
                        else:
                            b2_epilogue(Bslabs, h)

            def b1_epilogue(Bslabs, h):
                d2g = bpool.tile([P, G8 * 64], f32, tag="d2g")
                for j in range(G8):
                    nc.scalar.dma_start(
                        out=d2g[:, j * 64:(j + 1) * 64],
                        in_=D_row(h * G8 + j)[:, 64:128])
                rel = spool.tile([P, G8 * 64], f32, tag="rel")
                nc.scalar.activation(out=rel[:], in_=Bslabs[:], func=Relu)
                mn = spool.tile([P, G8 * 64], f32, tag="mn")
                nc.vector.tensor_scalar_min(mn[:], Bslabs[:], 0.0)
                exm = spool.tile([P, G8 * 64], f32, tag="exm")
                nc.scalar.activation(out=exm[:], in_=mn[:], func=Exp)
                qq = spool.tile([P, G8 * 64], f32, tag="qq")
                nc.vector.tensor_add(out=qq[:], in0=rel[:], in1=exm[:])
                h2st = stpool.tile([P, G8 * 64], f32, tag="h2st")
                for j in range(G8):
                    tps = pp.tile([64, 128], f32, space="PSUM", tag="tps")
                    nc.tensor.transpose(
                        out=tps[:], in_=qq[:, j * 64:(j + 1) * 64],
                        identity=ident)
                    qT = spool.tile([64, 128], f32, tag="qT")
                    nc.vector.tensor_copy(out=qT[:], in_=tps[:])
                    h2ps = pp.tile([P, 64], f32, space="PSUM", tag="h2ps")
                    nc.tensor.matmul(out=h2ps[:], lhsT=qT[:], rhs=w2sb,
                                     start=True, stop=True)
                    nc.vector.tensor_copy(
                        out=h2st[:, j * 64:(j + 1) * 64], in_=h2ps[:])
                den8 = spool.tile([P, G8 * 64], f32, tag="den8")
                nc.vector.tensor_scalar_add(den8[:], d2g[:], EPS)
                rec8 = spool.tile([P, G8 * 64], f32, tag="rec8")
                nc.vector.reciprocal(out=rec8[:], in_=den8[:])
                h2b = spool.tile([P, G8 * 64], f32, tag="h2b")
                nc.vector.tensor_add(
                    out=h2b[:].rearrange("p (j c) -> p j c", c=64),
                    in0=h2st[:].rearrange("p (j c) -> p j c", c=64),
                    in1=b2pb[:, None, :].to_broadcast([P, G8, 64]))
                p2st = stpool.tile([P, G8 * 64], f32, tag="p2st")
                nc.vector.tensor_mul(out=p2st[:], in0=h2b[:], in1=rec8[:])
                for j in range(G8):
                    nc.sync.dma_start(
                        out=P2_row(h * G8 + j),
                        in_=p2st[:, j * 64:(j + 1) * 64])

            def b2_epilogue(Bslabs, h):
                Bv = Bslabs[:].rearrange("p (j c) -> p j c", c=64)[:, :, 0:40]
                mx8 = spool.tile([P, G8], f32, tag="mx8")
                nc.vector.reduce_max(mx8[:], Bv, axis=X)
                xm8 = spool.tile([P, G8 * 40], f32, tag="xm8")
                nc.vector.tensor_tensor(
                    out=xm8[:].rearrange("p (j c) -> p j c", c=40),
                    in0=Bv, in1=mx8[:, :, None].to_broadcast([P, G8, 40]),
                    op=SUB)
                ex8 = spool.tile([P, G8 * 40], f32, tag="ex8")
                nc.scalar.activation(out=ex8[:], in_=xm8[:], func=Exp)
                sm8 = spool.tile([P, G8], f32, tag="sm8")
                nc.vector.reduce_sum(
                    sm8[:], ex8[:].rearrange("p (j c) -> p j c", c=40), axis=X)
                ln8 = spool.tile([P, G8], f32, tag="ln8")
                nc.scalar.activation(out=ln8[:], in_=sm8[:], func=Ln)
                ost = stpool.tile([P, G8 * 64], f32, tag="ost")
                nc.vector.memset(ost[:], 0.0)
                nc.vector.tensor_tensor(
                    out=ost[:].rearrange("p (j c) -> p j c", c=64)[:, :, 0:40],
                    in0=xm8[:].rearrange("p (j c) -> p j c", c=40),
                    in1=ln8[:, :, None].to_broadcast([P, G8, 40]), op=SUB)
                for j in range(G8):
                    nc.sync.dma_start(
                        out=out_rows[h * G8 + j],
                        in_=ost[:, j * 64:(j + 1) * 64])

            b_phase(P1full, u1b, layer=1)
            b_phase(P2full, u2b, layer=2)

            if debug:
                for c in range(NCH):
                    nc.sync.dma_start(
                        out=dbg["D"][:].rearrange("(h r) c -> h r c", r=CB * P)[c],
                        in_=D_ts[c][:])
                nc.sync.dma_start(out=dbg["P1"][:], in_=P1full[:])
                nc.sync.dma_start(out=dbg["P2"][:], in_=P2full[:])

    nc.compile()
    return nc


def _make_consts_array(pre):
    c = pre["consts"]
    arr = np.zeros((P, 680), np.float32)
    arr[:, 0:128] = np.arange(128, dtype=np.float32)[None, :]
    arr[:, 128:192] = c["u1"][None, :]
    arr[:, 192:256] = c["u2pad"][None, :]
    arr[:, 256:360] = c["ucat"][None, :]
    arr[:, 360:424] = c["b1"][None, :]
    arr[:, 424:488] = c["b2ppad"][None, :]
    arr[0:64, 488:552] = c["w2pad"]
    arr[:, 552:680] = np.eye(128, dtype=np.float32)
    return arr


def _in_maps(pre):
    carr = _make_consts_array(pre)
    w1c = pre["consts"]["w1"].astype(np.float32)            # [500, 64]
    w1c = w1c.reshape(4, 125, 64).copy()
    maps = []
    for core in pre["cores"]:
        maps.append({
            "xpre": core["xpre"],
            "wsegA": core["wsegA"],
            "wsegB": core["wsegB"],
            "gidx16": core["gidx16"],
            "consts": carr,
            "w1c": w1c,
        })
    return maps


def _install_ntff_hook():
    """Register the axon NTFF profiling hook (missing antenv.axon_hooks in
    this image). Best effort — profiling only."""
    import sys, types
    try:
        import antenv  # noqa: F401
        if "antenv.axon_hooks" not in sys.modules:
            mod = types.ModuleType("antenv.axon_hooks")
            holder = [None]
            mod.set_axon_ntff_profile_hook = lambda h: holder.__setitem__(0, h)
            mod.get_axon_ntff_profile_hook = lambda: holder[0]
            sys.modules["antenv.axon_hooks"] = mod
            from trn_agent_boot.trn_boot import _ntff_profile_via_ctypes
            mod.set_axon_ntff_profile_hook(
                _ntff_profile_via_ctypes("/opt/axon/libaxon_pjrt.so"))
    except Exception:
        pass


def _run(inputs, profile=False, debug=False):
    from concourse.bass_utils import run_bass_kernel_spmd
    if profile:
        _install_ntff_hook()
    pre = _preprocess(inputs)
    nc = _build_program(pre, debug=debug)
    maps = _in_maps(pre)
    res = run_bass_kernel_spmd(nc, maps, list(range(NCORES)), trace=profile)
    out = np.concatenate(
        [res.results[i]["out"][pre["cores"][i]["perm"], :N_CLS]
         for i in range(NCORES)], axis=0)
    return out.astype(np.float32), res


def kernel(**inputs):
    out, _ = _run(inputs)
    return out


# revision 11
# speedup vs baseline: 2.3509x; 1.1174x over previous
"""Bass/Trainium2 kernel for nn_Net_40063454937541 (CurvGN 2-layer GNN).

Strategy (8 NeuronCores, SPMD single program):
  - Node space [100000] split into 8 contiguous ranges of 12500.
  - Exploits w_mul >= 0: leaky_relu(w_mul @ ma) is exactly linear in w_mul, so
    pre-softmax edge logits are affine in w_mul and the bias cancels in the
    segment softmax: softmax weights = exp(w_e * u[c]) / sum_src exp(w * u[c]).
  - Per-core node RELABELING: local nodes are bin-packed (first-fit) into SB
    uniform bins of 128 rows subject to per-bin caps on phase-A edges (by src),
    per-class phase-B edges (by dst), and a mod-4 position constraint
    (row % 4 == node % 4) preserving the int16 gather class trick. All
    supertiles therefore cover static row ranges [128*s, 128*(s+1)) identical
    on every core, so segment-sum slabs / D-row loads are plain HWDGE
    dma_starts instead of gpsimd scatter/gather descriptors. Host applies the
    inverse permutation to the output.
  - Phase A (edges by src): per-source denominators D[104] for both layers via
    one-hot matmul segment sums, written directly to static D rows.
  - h1 phase overlaps phase A: h1 = x @ w1 + b1 into SBUF; after A completes,
    P1 = h1/(D1+eps) -> P1loc; AllGather -> replicated P1 table.
  - Phase B (edges by dst): dma_gather P1[src] (mod-4 stride trick, int16
    indices, 4 SWDGE queues), msg = exp(w*u1)*P1[src], one-hot matmul segment
    sum by dst; fused epilogue ELU -> @w2 -> /D2 -> P2; AllGather P2; second
    pass with u2 -> log_softmax -> output rows.
All floating point math runs on device; host only shards/packs indices and
folds the tiny (64-wide) weight MLPs.
"""

import numpy as np

N_NODES = 100000
N_EDGES = 1600000
N_FEAT = 500
HID = 64
N_CLS = 40

NCORES = 8
NLOC = N_NODES // NCORES          # 12500
P = 128
T_B = 16                          # edge tiles per B supertile (4 classes x 4)
CLS_TILES = 4                     # tiles per mod-4 class region
CAP_CLASS = CLS_TILES * P         # 512 edge slots per class region
T_A = 16                          # edge tiles per A supertile
CAP_A = T_A * P                   # 2048
G8 = 4                            # supertiles per epilogue batch (SB % G8 == 0)
SEG_PAD = 200.0                   # one-hot never matches
EPS = 1e-16


def _wrap16(vals, dtype=np.int16):
    """Layout an index vector [n] (n % 16 == 0) into the q7 wrapped form
    [128, n//16]: position i lives at [i % 16, i // 16], replicated in all
    8 groups of 16 partitions."""
    v = np.asarray(vals)
    n = v.shape[0]
    assert n % 16 == 0
    w = v.reshape(n // 16, 16).T.astype(dtype)     # [16, n//16]
    return np.tile(w, (8, 1))                      # [128, n//16]


def _pack_bins(a_cnt, b_cnt, n_bins):
    """Worst-fit (load-balancing) node packing into n_bins uniform bins of
    128 positions. Constraints per bin: sum a_cnt <= CAP_A; per class r:
    sum b_cnt[r] <= CAP_CLASS; <= 32 nodes per residue class (position p
    holds only nodes with node % 4 == p % 4). Returns perm[n] -> row
    (bin*128 + pos) or None if infeasible."""
    n = len(a_cnt)
    order = np.argsort(-(a_cnt / CAP_A +
                         (b_cnt / CAP_CLASS).sum(axis=0)), kind="stable")
    a_used = np.zeros(n_bins)
    b_used = np.zeros((4, n_bins))
    res_used = np.zeros((4, n_bins), dtype=np.int64)
    perm = np.full(n, -1, dtype=np.int64)
    for node in order:
        r = node % 4
        na = a_used + a_cnt[node]
        nb = b_used + b_cnt[:, node][:, None]
        nr = res_used[r] + 1
        ok = (nr <= 32) & (na <= CAP_A) & np.all(nb <= CAP_CLASS, axis=0)
        if not ok.any():
            return None
        score = np.maximum((nb / CAP_CLASS).max(axis=0),
                           np.maximum(na / CAP_A, nr / 32.0))
        score[~ok] = 9e9
        s = int(np.argmin(score))
        a_used[s] = na[s]
        b_used[:, s] = nb[:, s]
        perm[node] = s * P + (res_used[r, s] * 4 + r)
        res_used[r, s] = nr[s]
    return perm


def _fold_weights(m1a, m1b_w, m1b_b, m2a, m2b_w, m2b_b, w2, b2):
    """Fold the tiny weight-MLPs using w_mul >= 0 (leaky_relu linear in w)."""
    s1 = np.where(m1a[0] >= 0, m1a[0], 0.2 * m1a[0])   # [64]
    u1 = (s1 @ m1b_w).astype(np.float32)               # [64]
    s2 = np.where(m2a[0] >= 0, m2a[0], 0.2 * m2a[0])   # [40]
    u2 = (s2 @ m2b_w).astype(np.float32)               # [40]
    # ELU fold: elu(x) = relu(x) + exp(min(x,0)) - 1; (q-1)@w2+b2 = q@w2+b2p
    b2p = (b2 - w2.sum(axis=0)).astype(np.float32)     # [40]
    return u1, u2, b2p


def _preprocess(inputs):
    """Build all per-core host arrays. Integer/layout work only (plus the
    tiny 64-wide weight folds)."""
    src = np.asarray(inputs["edge_index"][0], dtype=np.int64)
    dst = np.asarray(inputs["edge_index"][1], dtype=np.int64)
    w = np.asarray(inputs["w_mul"], dtype=np.float32).reshape(-1)
    x = np.asarray(inputs["x"], dtype=np.float32)

    u1, u2, b2p = _fold_weights(
        np.asarray(inputs["m1a"], np.float32), np.asarray(inputs["m1b_w"], np.float32),
        np.asarray(inputs["m1b_b"], np.float32), np.asarray(inputs["m2a"], np.float32),
        np.asarray(inputs["m2b_w"], np.float32), np.asarray(inputs["m2b_b"], np.float32),
        np.asarray(inputs["w2"], np.float32), np.asarray(inputs["b2"], np.float32))
    assert np.abs(u1).max() < 8 and np.abs(u2).max() < 8

    ucat = np.concatenate([u1, u2])                    # [104]
    u2pad = np.zeros(64, np.float32)
    u2pad[:N_CLS] = u2
    w2pad = np.zeros((64, 64), np.float32)
    w2pad[:, :N_CLS] = np.asarray(inputs["w2"], np.float32)
    b2ppad = np.zeros(64, np.float32)
    b2ppad[:N_CLS] = b2p

    core_shard = src // NLOC                           # phase A owner
    dst_shard = dst // NLOC                            # phase B owner
    cls = (src % 4).astype(np.int64)

    # per-core per-node counts
    a_cnts, b_cnts = [], []
    for i in range(NCORES):
        lo = i * NLOC
        m = core_shard == i
        a_cnts.append(np.bincount(src[m] - lo, minlength=NLOC))
        m = dst_shard == i
        b_cnts.append(np.stack([
            np.bincount(dst[m & (cls == r)] - lo, minlength=NLOC)
            for r in range(4)]))

    # find a bin count that packs every core
    SB = 80
    perms = None
    while perms is None:
        SB += 20              # SB % 20 == 0: 5 AllGather chunks of G8-aligned bins
        assert SB <= 140, "node bin packing failed"
        trial = [_pack_bins(a_cnts[i], b_cnts[i], SB) for i in range(NCORES)]
        if all(p is not None for p in trial):
            perms = trial
    NROWS = SB * P

    # global table row of node n — chunk-major layout so each AllGather
    # chunk writes one contiguous block: row = chunk*(8*CBP) + core*CBP + pos
    NCH = 5
    assert SB % NCH == 0 and (SB // NCH) % G8 == 0
    CBP = (SB // NCH) * P
    tab_row = np.empty(N_NODES, dtype=np.int64)
    for i in range(NCORES):
        pr = perms[i]
        tab_row[i * NLOC:(i + 1) * NLOC] = (
            (pr // CBP) * (NCORES * CBP) + i * CBP + (pr % CBP))
    assert np.all((tab_row % 4) == (np.arange(N_NODES) % 4))

    cores = []
    for i in range(NCORES):
        lo = i * NLOC
        perm = perms[i]
        core = {"perm": perm}

        # ---------- phase A (by src, rows = perm[src-lo]) ----------
        m = core_shard == i
        ar, aw = perm[src[m] - lo], w[m]
        order = np.argsort(ar, kind="stable")
        ar, aw = ar[order], aw[order]
        wA = np.zeros((SB, P, T_A), np.float32)
        segA = np.full((SB, P, T_A), SEG_PAD, np.float32)
        sA = ar // P
        bnd = np.searchsorted(sA, np.arange(SB + 1))
        for s in range(SB):
            e0, e1 = bnd[s], bnd[s + 1]
            ne = e1 - e0
            assert ne <= CAP_A
            q = np.arange(ne)
            kk, pp = q // P, q % P
            wA[s, pp, kk] = aw[e0:e1]
            segA[s, pp, kk] = (ar[e0:e1] - s * P).astype(np.float32)
        core["wsegA"] = np.concatenate([wA, segA], axis=2)  # [SB,128,32]

        # ---------- phase B (by dst rows, classes by src%4) ----------
        m = dst_shard == i
        br, bg, bw, bc = perm[dst[m] - lo], src[m], w[m], cls[m]
        wB = np.zeros((SB, P, T_B), np.float32)
        segB = np.full((SB, P, T_B), SEG_PAD, np.float32)
        gidx = np.zeros((SB, 4, CAP_CLASS), np.int64)   # per supertile+class
        sB = br // P
        for r in range(4):
            mr = bc == r
            rr, rg, rw = br[mr], bg[mr], bw[mr]
            order = np.argsort(rr // P, kind="stable")
            rr, rg, rw = rr[order], rg[order], rw[order]
            bnd = np.searchsorted(rr // P, np.arange(SB + 1))
            for s in range(SB):
                a, b = bnd[s], bnd[s + 1]
                ne = b - a
                assert ne <= CAP_CLASS
                q = np.arange(ne)
                kk, pp = q // P, q % P
                wB[s, pp, 4 * r + kk] = rw[a:b]
                segB[s, pp, 4 * r + kk] = (rr[a:b] - s * P).astype(np.float32)
                gidx[s, r, :ne] = tab_row[rg[a:b]] // 4
        core["wsegB"] = np.concatenate([wB, segB], axis=2)  # [SB,128,32]
        core["gidxB"] = gidx

        # ---------- h1 phase: x pretransposed into permuted rows ----------
        xpad = np.zeros((NROWS, N_FEAT), np.float32)
        xpad[perm] = x[lo:lo + NLOC]
        xp = np.zeros((SB, 4, 125, P), np.float32)
        for j in range(SB):
            xp[j] = xpad[j * P:(j + 1) * P].T.reshape(4, 125, P)
        core["xpre"] = xp
        cores.append(core)

    # wrapped int16 gather indices, per pair of supertiles
    for c in cores:
        gi = c["gidxB"]                                    # [SB,4,512]
        pairs = gi.reshape(SB // 2, 2, 4, CAP_CLASS).transpose(0, 2, 1, 3)
        pairs = pairs.reshape(SB // 2, 4, 2 * CAP_CLASS)   # [G2,4,1024]
        c["gidx16"] = np.stack([
            np.concatenate([_wrap16(pairs[g, r]) for r in range(4)], axis=1)
            for g in range(SB // 2)])                      # [G2,128,256] i16

    consts = {
        "u1": u1, "u2pad": u2pad, "ucat": ucat, "w2pad": w2pad,
        "b2ppad": b2ppad,
        "b1": np.asarray(inputs["b1"], np.float32),
        "w1": np.asarray(inputs["w1"], np.float32),
    }
    return {"cores": cores, "SB": SB, "tab_row": tab_row, "consts": consts}


def _emulate(pre, inputs):
    """Numpy emulation of the exact device dataflow (for validation)."""
    consts = pre["consts"]
    u1, u2pad, ucat = consts["u1"], consts["u2pad"], consts["ucat"]
    w1, b1 = consts["w1"], consts["b1"]
    w2pad, b2ppad = consts["w2pad"], consts["b2ppad"]
    SB = pre["SB"]
    NROWS = SB * P
    x = np.asarray(inputs["x"], np.float32)

    def segsum(wseg_s, vals):
        """vals [128, T, C] -> slab [128, C] summed by seg id."""
        seg = wseg_s[:, 16:]
        segf = seg.reshape(-1).astype(np.int64)
        vf = vals.reshape(-1, vals.shape[-1])
        valid = segf < P
        slab = np.zeros((P, vals.shape[-1]), np.float32)
        np.add.at(slab, segf[valid], vf[valid])
        return slab

    D = []
    for c in pre["cores"]:
        Di = np.zeros((NROWS, 128), np.float32)
        for s in range(SB):
            w_ = c["wsegA"][s, :, :16]
            ex = np.exp(w_[:, :, None] * ucat[None, None, :])
            slab = segsum(c["wsegA"][s], ex)
            Di[s * P:(s + 1) * P, :104] = slab
        D.append(Di)

    tab = pre["tab_row"]
    P1full = np.zeros((NCORES * NROWS, 64), np.float32)
    for i, c in enumerate(pre["cores"]):
        xpad = np.zeros((NROWS, N_FEAT), np.float32)
        xpad[c["perm"]] = x[i * NLOC:(i + 1) * NLOC]
        h1 = xpad @ w1 + b1
        P1loc = h1 / (D[i][:, :64] + EPS)
        P1full[tab[i * NLOC:(i + 1) * NLOC]] = P1loc[c["perm"]]

    def b_phase(core, table, u):
        gath = np.zeros((SB, P, T_B, 64), np.float32)
        for s in range(SB):
            for r in range(4):
                rows = core["gidxB"][s, r] * 4 + r            # [512]
                g = table[rows]                               # [512, 64]
                q = np.arange(CAP_CLASS)
                gath[s, q % P, 4 * r + q // P] = g
        slabs = []
        for s in range(SB):
            w_ = core["wsegB"][s, :, :16]
            ex = np.exp(w_[:, :, None] * u[None, None, :])
            msg = ex * gath[s]
            slabs.append(segsum(core["wsegB"][s], msg))
        return slabs

    P2full = np.zeros((NCORES * NROWS, 64), np.float32)
    for i, c in enumerate(pre["cores"]):
        slabs = b_phase(c, P1full, u1)
        P2loc = np.zeros((NROWS, 64), np.float32)
        for s in range(SB):
            o1 = slabs[s]
            q_ = np.maximum(o1, 0) + np.exp(np.minimum(o1, 0))
            h2 = q_ @ w2pad + b2ppad
            p2 = h2 * (1.0 / (D[i][s * P:(s + 1) * P, 64:128] + EPS))
            P2loc[s * P:(s + 1) * P] = p2
        P2full[tab[i * NLOC:(i + 1) * NLOC]] = P2loc[c["perm"]]

    out = np.zeros((N_NODES, N_CLS), np.float32)
    for i, c in enumerate(pre["cores"]):
        slabs = b_phase(c, P2full, u2pad)
        OUT = np.zeros((NROWS, 64), np.float32)
        for s in range(SB):
            o2 = slabs[s][:, :N_CLS]
            m = o2.max(axis=1, keepdims=True)
            e = np.exp(o2 - m)
            ls = (o2 - m) - np.log(e.sum(axis=1, keepdims=True))
            OUT[s * P:(s + 1) * P, :N_CLS] = ls
        out[i * NLOC:(i + 1) * NLOC] = OUT[c["perm"], :N_CLS]
    return out


# ---------------------------------------------------------------------------
# device program
# ---------------------------------------------------------------------------

def _build_program(pre, debug=False):
    import concourse.bacc as bacc
    import concourse.mybir as mybir
    import concourse.tile as tile

    SB = pre["SB"]
    NROWS = SB * P
    G2 = SB // 2
    NCH = 5                        # AllGather chunks
    assert SB % NCH == 0 and (SB // NCH) % G8 == 0
    CB = SB // NCH                 # bins per chunk
    f32 = mybir.dt.float32
    bf16 = mybir.dt.bfloat16
    i16 = mybir.dt.int16
    Exp = mybir.ActivationFunctionType.Exp
    Ln = mybir.ActivationFunctionType.Ln
    Relu = mybir.ActivationFunctionType.Relu
    X = mybir.AxisListType.X
    EQ = mybir.AluOpType.is_equal
    SUB = mybir.AluOpType.subtract
    MUL = mybir.AluOpType.mult

    nc = bacc.Bacc("TRN2", target_bir_lowering=False, debug=False,
                   num_devices=NCORES, num_swdge_queues=4)

    xpre_d = nc.declare_dram_parameter("xpre", [SB, 4, 125, P], f32, isOutput=False)
    wsegA_d = nc.declare_dram_parameter("wsegA", [SB, P, 32], f32, isOutput=False)
    wsegB_d = nc.declare_dram_parameter("wsegB", [SB, P, 32], f32, isOutput=False)
    gidx_d = nc.declare_dram_parameter("gidx16", [G2, P, 256], i16, isOutput=False)
    consts_d = nc.declare_dram_parameter("consts", [P, 680], f32, isOutput=False)
    w1_d = nc.declare_dram_parameter("w1c", [4, 125, 64], f32, isOutput=False)
    out_d = nc.declare_dram_parameter("out", [NROWS, 64], f32, isOutput=True)
    dbg = {}
    if debug:
        dbg["D"] = nc.declare_dram_parameter("dbg_D", [NROWS, 128], f32, isOutput=True)
        dbg["P1"] = nc.declare_dram_parameter("dbg_P1", [NCORES * NROWS, 64], f32, isOutput=True)
        dbg["P2"] = nc.declare_dram_parameter("dbg_P2", [NCORES * NROWS, 64], f32, isOutput=True)

    with tile.TileContext(nc) as tc:
        with (
            tc.tile_pool(name="cpool", bufs=1) as cpool,
            tc.tile_pool(name="dram", bufs=1, space="DRAM") as dpool,
            tc.tile_pool(name="big", bufs=3) as bpool,
            tc.tile_pool(name="small", bufs=3) as spool,
            tc.tile_pool(name="stage", bufs=2) as stpool,
            tc.tile_pool(name="psum", bufs=2, space="PSUM") as pp,
        ):
            # ---- DRAM internals (per-chunk tiles so collective deps are
            # exact even under coarse whole-tile tracking) ----
            D_ts = [dpool.tile([CB * P, 128], f32, tag=f"D{c}", name=f"D{c}")
                    for c in range(NCH)]
            P1locs = [dpool.tile([CB * P, 64], f32, tag=f"P1loc{c}", name=f"P1loc{c}")
                      for c in range(NCH)]
            P2locs = [dpool.tile([CB * P, 64], f32, tag=f"P2loc{c}", name=f"P2loc{c}")
                      for c in range(NCH)]
            P1full = dpool.tile([NCORES * NROWS, 64], f32, tag="P1full")
            P2full = dpool.tile([NCORES * NROWS, 64], f32, tag="P2full")

            def D_row(j):
                return D_ts[j // CB][:].rearrange("(s p) c -> s p c", p=P)[j % CB]

            def P1_row(j):
                return P1locs[j // CB][:].rearrange("(s p) c -> s p c", p=P)[j % CB]

            def P2_row(j):
                return P2locs[j // CB][:].rearrange("(s p) c -> s p c", p=P)[j % CB]

            out_rows = out_d[:].rearrange("(s p) c -> s p c", p=P)

            # ---- constants ----
            consts = cpool.tile([P, 680], f32, tag="consts")
            nc.sync.dma_start(out=consts[:], in_=consts_d[:])
            iota_t = consts[:, 0:128]
            u1b = consts[:, 128:192]
            u2b = consts[:, 192:256]
            ucatb = consts[:, 256:360]
            b1b = consts[:, 360:424]
            b2pb = consts[:, 424:488]
            w2sb = consts[0:64, 488:552]
            ident = consts[:, 552:680]
            w1sb = cpool.tile([125, 256], f32, tag="w1sb")
            for c in range(4):
                nc.sync.dma_start(out=w1sb[:, c * 64:(c + 1) * 64], in_=w1_d[c])

            # h1 staging (SBUF-resident so the x@w1 matmuls overlap phase A)
            h1sb = cpool.tile([P, SB * 64], f32, tag="h1sb")

            # ---- phase A + h1 + P1 divide + chunked AllGather ----
            for ch in range(NCH):
                for s in range(ch * CB, (ch + 1) * CB):
                    meta = spool.tile([P, 32], f32, tag="metaA")
                    nc.sync.dma_start(out=meta[:], in_=wsegA_d[s])
                    prodA = bpool.tile([P, T_A * 104], f32, tag="prodA")
                    nc.vector.tensor_tensor(
                        out=prodA[:].rearrange("p (t c) -> p t c", t=T_A),
                        in0=meta[:, 0:16, None].to_broadcast([P, T_A, 104]),
                        in1=ucatb[:, None, :].to_broadcast([P, T_A, 104]), op=MUL)
                    ex = bpool.tile([P, T_A * 104], bf16, tag="exA")
                    nc.scalar.activation(out=ex[:], in_=prodA[:], func=Exp)
                    oh = bpool.tile([P, T_A * P], bf16, tag="oh")
                    nc.vector.tensor_tensor(
                        out=oh[:].rearrange("p (t q) -> p t q", t=T_A),
                        in0=meta[:, 16:32, None].to_broadcast([P, T_A, P]),
                        in1=iota_t[:, None, :].to_broadcast([P, T_A, P]), op=EQ)
                    ps = pp.tile([P, 104], f32, space="PSUM", tag="ps")
                    for k in range(T_A):
                        nc.tensor.matmul(
                            out=ps[:], lhsT=oh[:, k * P:(k + 1) * P],
                            rhs=ex[:, k * 104:(k + 1) * 104],
                            start=(k == 0), stop=(k == T_A - 1))
                    At = stpool.tile([P, 128], f32, tag="At")
                    nc.vector.tensor_copy(out=At[:, 0:104], in_=ps[:])
                    nc.vector.memset(At[:, 104:128], 0.0)
                    nc.sync.dma_start(out=D_row(s), in_=At[:])

                for j in range(ch * CB, (ch + 1) * CB):
                    xt = spool.tile([125, 512], f32, tag="xt")
                    nc.scalar.dma_start(
                        out=xt[:].rearrange("p (c f) -> p c f", c=4),
                        in_=xpre_d[j].rearrange("c p f -> p c f"))
                    hps = pp.tile([P, 64], f32, space="PSUM", tag="hps")
                    for c in range(4):
                        nc.tensor.matmul(
                            out=hps[:], lhsT=xt[:, c * 128:(c + 1) * 128],
                            rhs=w1sb[:, c * 64:(c + 1) * 64],
                            start=(c == 0), stop=(c == 3))
                    nc.vector.tensor_add(
                        out=h1sb[:, j * 64:(j + 1) * 64], in0=hps[:], in1=b1b)

                for j in range(ch * CB, (ch + 1) * CB):
                    d1 = spool.tile([P, 64], f32, tag="d1")
                    nc.scalar.dma_start(out=d1[:], in_=D_row(j)[:, 0:64])
                    nc.vector.tensor_scalar_add(d1[:], d1[:], EPS)
                    rc = spool.tile([P, 64], f32, tag="rc")
                    nc.vector.reciprocal(out=rc[:], in_=d1[:])
                    p1t = spool.tile([P, 64], f32, tag="p1t")
                    nc.vector.tensor_mul(
                        out=p1t[:], in0=h1sb[:, j * 64:(j + 1) * 64], in1=rc[:])
                    nc.sync.dma_start(out=P1_row(j), in_=p1t[:])

                nc.gpsimd.collective_compute(
                    "AllGather", mybir.AluOpType.bypass,
                    replica_groups=[list(range(NCORES))],
                    ins=[P1locs[ch][:].opt()],
                    outs=[P1full[ch * NCORES * CB * P:
                                 (ch + 1) * NCORES * CB * P].opt()])

            # ---- phase B (shared) ----
            def b_phase(table, u_ap, layer):
                for s in range(SB):
                    g2, half = s // 2, s % 2
                    if half == 0:
                        gpair = bpool.tile([P, 32 * 64], f32, tag="gpair")
                        gix = spool.tile([P, 256], i16, tag="gix")
                        nc.sync.dma_start(out=gix[:], in_=gidx_d[g2])
                        tview = table[:].rearrange("(q f) c -> q (f c)", f=4)
                        for r in range(4):
                            nc.gpsimd.dma_gather(
                                out_ap=gpair[:, r * 512:(r + 1) * 512]
                                    .rearrange("p (t c) -> p t c", c=64),
                                in_ap=tview[:, r * 64:(r + 1) * 64],
                                idxs_ap=gix[:, r * 64:(r + 1) * 64],
                                num_idxs=1024, num_idxs_reg=1024,
                                elem_size=64, elem_step=256,
                                queue_num=r)
                    meta = spool.tile([P, 32], f32, tag="metaB")
                    nc.sync.dma_start(out=meta[:], in_=wsegB_d[s])
                    prodB = bpool.tile([P, 1024], f32, tag="prodB")
                    nc.vector.tensor_tensor(
                        out=prodB[:].rearrange("p (t c) -> p t c", t=T_B),
                        in0=meta[:, 0:16, None].to_broadcast([P, T_B, 64]),
                        in1=u_ap[:, None, :].to_broadcast([P, T_B, 64]), op=MUL)
                    ex = bpool.tile([P, 1024], f32, tag="exB")
                    nc.scalar.activation(out=ex[:], in_=prodB[:], func=Exp)
                    msg = bpool.tile([P, 1024], bf16, tag="msg")
                    gv = gpair[:].rearrange("p (r h c) -> p r h c", r=4, h=2)[:, :, half, :]
                    nc.vector.tensor_tensor(
                        out=msg[:].rearrange("p (r c) -> p r c", r=4),
                        in0=ex[:].rearrange("p (r c) -> p r c", r=4),
                        in1=gv, op=MUL)
                    oh = bpool.tile([P, T_B * P], bf16, tag="oh")
                    nc.vector.tensor_tensor(
                        out=oh[:].rearrange("p (t q) -> p t q", t=T_B),
                        in0=meta[:, 16:32, None].to_broadcast([P, T_B, P]),
                        in1=iota_t[:, None, :].to_broadcast([P, T_B, P]), op=EQ)
                    ps = pp.tile([P, 104], f32, space="PSUM", tag="ps")
                    for k in range(T_B):
                        nc.tensor.matmul(
                            out=ps[:, 0:64], lhsT=oh[:, k * P:(k + 1) * P],
                            rhs=msg[:, k * 64:(k + 1) * 64],
                            start=(k == 0), stop=(k == T_B - 1))
                    s8 = s % G8
                    if s8 == 0:
                        Bslabs = stpool.tile([P, G8 * 64], f32, tag="Bslabs")
                    nc.vector.tensor_copy(
                        out=Bslabs[:, s8 * 64:(s8 + 1) * 64], in_=ps[:, 0:64])
                    if s8 == G8 - 1:
                        h = s // G8
                        if layer == 1:
                            b1_epilogue(Bslabs, h)
                            if (s + 1) % CB == 0:
                                ch = (s + 1) // CB - 1
                                nc.gpsimd.collective_compute(
                                    "AllGather", mybir.AluOpType.bypass,
                                    replica_groups=[list(range(NCORES))],
                                    ins=[P2locs[ch][:].opt()],
                                    outs=[P2full[ch * NCORES * CB * P:
                                                 (ch + 1) * NCORES * CB * P].opt()])
                        else:
                            b2_epilogue(Bslabs, h)

            def b1_epilogue(Bslabs, h):
                d2g = bpool.tile([P, G8 * 64], f32, tag="d2g")
                for j in range(G8):
                    nc.scalar.dma_start(
                        out=d2g[:, j * 64:(j + 1) * 64],
                        in_=D_row(h * G8 + j)[:, 64:128])
                rel = spool.tile([P, G8 * 64], f32, tag="rel")
                nc.scalar.activation(out=rel[:], in_=Bslabs[:], func=Relu)
                mn = spool.tile([P, G8 * 64], f32, tag="mn")
                nc.vector.tensor_scalar_min(mn[:], Bslabs[:], 0.0)
                exm = spool.tile([P, G8 * 64], f32, tag="exm")
                nc.scalar.activation(out=exm[:], in_=mn[:], func=Exp)
                qq = spool.tile([P, G8 * 64], f32, tag="qq")
                nc.vector.tensor_add(out=qq[:], in0=rel[:], in1=exm[:])
                h2st = stpool.tile([P, G8 * 64], f32, tag="h2st")
                for j in range(G8):
                    tps = pp.tile([64, 128], f32, space="PSUM", tag="tps")
                    nc.tensor.transpose(
                        out=tps[:], in_=qq[:, j * 64:(j + 1) * 64],
                        identity=ident)
                    qT = spool.tile([64, 128], f32, tag="qT")
                    nc.vector.tensor_copy(out=qT[:], in_=tps[:])
                    h2ps = pp.tile([P, 64], f32, space="PSUM", tag="h2ps")
                    nc.tensor.matmul(out=h2ps[:], lhsT=qT[:], rhs=w2sb,
                                     start=True, stop=True)
                    nc.vector.tensor_copy(
                        out=h2st[:, j * 64:(j + 1) * 64], in_=h2ps[:])
                den8 = spool.tile([P, G8 * 64], f32, tag="den8")
                nc.vector.tensor_scalar_add(den8[:], d2g[:], EPS)
                rec8 = spool.tile([P, G8 * 64], f32, tag="rec8")
                nc.vector.reciprocal(out=rec8[:], in_=den8[:])
                h2b = spool.tile([P, G8 * 64], f32, tag="h2b")
                nc.vector.tensor_add(
                    out=h2b[:].rearrange("p (j c) -> p j c", c=64),
                    in0=h2st[:].rearrange("p (j c) -> p j c", c=64),
                    in1=b2pb[:, None, :].to_broadcast([P, G8, 64]))
                p2st = stpool.tile([P, G8 * 64], f32, tag="p2st")
                nc.vector.tensor_mul(out=p2st[:], in0=h2b[:], in1=rec8[:])
                for j in range(G8):
                    nc.sync.dma_start(
                        out=P2_row(h * G8 + j),
                        in_=p2st[:, j * 64:(j + 1) * 64])

            def b2_epilogue(Bslabs, h):
                Bv = Bslabs[:].rearrange("p (j c) -> p j c", c=64)[:, :, 0:40]
                mx8 = spool.tile([P, G8], f32, tag="mx8")
                nc.vector.reduce_max(mx8[:], Bv, axis=X)
                xm8 = spool.tile([P, G8 * 40], f32, tag="xm8")
                nc.vector.tensor_tensor(
                    out=xm8[:].rearrange("p (j c) -> p j c", c=40),
                    in0=Bv, in1=mx8[:, :, None].to_broadcast([P, G8, 40]),
                    op=SUB)
                ex8 = spool.tile([P, G8 * 40], f32, tag="ex8")
                nc.scalar.activation(out=ex8[:], in_=xm8[:], func=Exp)
                sm8 = spool.tile([P, G8], f32, tag="sm8")
                nc.vector.reduce_sum(
                    sm8[:], ex8[:].rearrange("p (j c) -> p j c", c=40), axis=X)
                ln8 = spool.tile([P, G8], f32, tag="ln8")
                nc.scalar.activation(out=ln8[:], in_=sm8[:], func=Ln)
                ost = stpool.tile([P, G8 * 64], f32, tag="ost")
                nc.vector.memset(ost[:], 0.0)
                nc.vector.tensor_tensor(
                    out=ost[:].rearrange("p (j c) -> p j c", c=64)[:, :, 0:40],
                    in0=xm8[:].rearrange("p (j c) -> p j c", c=40),
                    in1=ln8[:, :, None].to_broadcast([P, G8, 40]), op=SUB)
                for j in range(G8):
                    nc.sync.dma_start(
                        out=out_rows[h * G8 + j],
                        in_=ost[:, j * 64:(j + 1) * 64])

            b_phase(P1full, u1b, layer=1)
            b_phase(P2full, u2b, layer=2)

            if debug:
                for c in range(NCH):
                    nc.sync.dma_start(
                        out=dbg["D"][:].rearrange("(h r) c -> h r c", r=CB * P)[c],
                        in_=D_ts[c][:])
                nc.sync.dma_start(out=dbg["P1"][:], in_=P1full[:])
                nc.sync.dma_start(out=dbg["P2"][:], in_=P2full[:])

    nc.compile()
    return nc


def _make_consts_array(pre):
    c = pre["consts"]
    arr = np.zeros((P, 680), np.float32)
    arr[:, 0:128] = np.arange(128, dtype=np.float32)[None, :]
    arr[:, 128:192] = c["u1"][None, :]
    arr[:, 192:256] = c["u2pad"][None, :]
    arr[:, 256:360] = c["ucat"][None, :]
    arr[:, 360:424] = c["b1"][None, :]
    arr[:, 424:488] = c["b2ppad"][None, :]
    arr[0:64, 488:552] = c["w2pad"]
    arr[:, 552:680] = np.eye(128, dtype=np.float32)
    return arr


def _in_maps(pre):
    carr = _make_consts_array(pre)
    w1c = pre["consts"]["w1"].astype(np.float32)            # [500, 64]
    w1c = w1c.reshape(4, 125, 64).copy()
    maps = []
    for core in pre["cores"]:
        maps.append({
            "xpre": core["xpre"],
            "wsegA": core["wsegA"],
            "wsegB": core["wsegB"],
            "gidx16": core["gidx16"],
            "consts": carr,
            "w1c": w1c,
        })
    return maps


def _install_ntff_hook():
    """Register the axon NTFF profiling hook (missing antenv.axon_hooks in
    this image). Best effort — profiling only."""
    import sys, types
    try:
        import antenv  # noqa: F401
        if "antenv.axon_hooks" not in sys.modules:
            mod = types.ModuleType("antenv.axon_hooks")
            holder = [None]
            mod.set_axon_ntff_profile_hook = lambda h: holder.__setitem__(0, h)
            mod.get_axon_ntff_profile_hook = lambda: holder[0]
            sys.modules["antenv.axon_hooks"] = mod
            from trn_agent_boot.trn_boot import _ntff_profile_via_ctypes
            mod.set_axon_ntff_profile_hook(
                _ntff_profile_via_ctypes("/opt/axon/libaxon_pjrt.so"))
    except Exception:
        pass


def _run(inputs, profile=False, debug=False):
    from concourse.bass_utils import run_bass_kernel_spmd
    if profile:
        _install_ntff_hook()
    pre = _preprocess(inputs)
    nc = _build_program(pre, debug=debug)
    maps = _in_maps(pre)
    res = run_bass_kernel_spmd(nc, maps, list(range(NCORES)), trace=profile)
    out = np.concatenate(
        [res.results[i]["out"][pre["cores"][i]["perm"], :N_CLS]
         for i in range(NCORES)], axis=0)
    return out.astype(np.float32), res


def kernel(**inputs):
    out, _ = _run(inputs)
    return out
